# revision 19
# baseline (speedup 1.0000x reference)
"""Multi-head causal attention (B=1024, T=64, C=768, H=12, D=64) on 8 TRN2
NeuronCores, data-parallel over the batch dimension (128 batches/core).

v2 dataflow per core (1024-token chunks, TT=8 128-token tiles, NB=16 batches):
  - All transposes ride the XBAR DMA engine (dma_start_transpose, 14ns per
    16x128 tile): X tiles, Y tiles and all four weight preps. The PE runs
    only "real" matmuls.
  - Q/K projections run in fp8(e4m3) DoubleRow perf mode (2 contraction
    rows/partition/cycle): W is pre-scaled by 2^10 into fp8, X is cast
    straight to fp8 (validated: final rel err ~1.6e-2 < 2e-2 gate); the
    2^-20 descale is folded into the softmax exp scale. V / output
    projections stay bf16.
  - scores+exp and the V projection interleave per tile so the Scalar exp
    drains while the PE streams V; AV / normalize / XBAR-Y / output
    projection software-pipeline per tile as before.
  - Evacuations are spread across engines: q+mask+normalize+recip+x-fp8
    casts on DVE, k+v_sb+exp on Scalar, out-bias add on GpSimd, XBARs on
    the SP/Activation HWDGE queues.
"""

import numpy as np

P = 128
B, T, C, H, Dh = 1024, 64, 768, 12, 64
HD = H * Dh            # 768
NCC = C // P           # 6 contraction chunks
NHD = HD // P          # 6 hd chunks
N_CORES = 8
F8CC = 3               # how many 256-channel pairs of the QK contraction run fp8
                       # (3 = all 768 channels; 2 = 512 fp8 + 256 bf16 fallback)

_cache = {}


def _patch_tile_drain(tile, mybir):
    """walrus CTRL (Drain) ops in this toolchain accept only 1 sem-wait;
    spread the TileContext exit-drain's waits across preceding SP nops."""
    from concourse.vector_clock import ScopedClock

    if getattr(tile.TileContext, "_drain_patched", False):
        return

    def _drain_and_barrier(self, tick_clock, wait_clock):
        nc = self.nc
        drain_inst = nc.sync.drain()
        wait_clock.add_sem_waits(
            drain_inst.ins, ScopedClock({None: tick_clock.global_clock})
        )
        waits = list(drain_inst.ins.sync_info.on_wait)
        if len(waits) > 1:
            drain_inst.ins.sync_info.on_wait = waits[-1:]
            cur_bb = nc.cur_bb.bb
            idx = cur_bb.instructions.index(drain_inst.ins)
            extra = []
            for w in waits[:-1]:
                nop = mybir.InstNoOp(name=f"I-{nc.next_id()}", ins=[], outs=[])
                nop.engine = drain_inst.ins.engine
                nop.sync_info = mybir.SyncInfo(on_wait=[w], on_update=[])
                nc.register_instruction(nop)
                extra.append(nop)
            cur_bb.instructions[idx:idx] = extra
        nc.all_engine_barrier()
        assert self.sems is not None
        popped = nc._tile_sem_poison_stack.pop()
        assert popped is self._sem_poison
        nc.clear_and_free_semaphores(list(self.sems.allocated().values()))
        nc.all_engine_barrier()

    tile.TileContext._drain_and_barrier = _drain_and_barrier
    tile.TileContext._drain_patched = True


def _install_loud_cc_hook():
    """Surface real exceptions from the neuronx_cc hook (C wrapper eats them)."""
    from concourse import bass2jax as _b2j
    if getattr(_b2j, "_loud_hook_installed", False):
        return
    _orig = _b2j.neuronx_cc_hook
    def _loud(*a, **k):
        try:
            return _orig(*a, **k)
        except BaseException:
            import traceback
            traceback.print_exc()
            raise
    _b2j.neuronx_cc_hook = _loud
    _b2j._loud_hook_installed = True


def _split_multi_waits(nc, mybir, K=1):
    """This walrus build supports only one sem-wait per instruction: move
    excess waits onto same-engine NOPs inserted directly before the owner."""
    def fix_block(bb):
        insts = bb.instructions
        i = 0
        while i < len(insts):
            ins = insts[i]
            si = ins.sync_info
            w = list(si.on_wait) if si is not None and si.on_wait else []
            if len(w) > K:
                carriers = []
                for j in range(0, len(w) - K, K):
                    nop = mybir.InstNoOp(name=f"I-{nc.next_id()}", ins=[], outs=[])
                    nop.engine = ins.engine
                    nop.sync_info = mybir.SyncInfo(on_wait=w[j:j + K], on_update=[])
                    nc.register_instruction(nop)
                    carriers.append(nop)
                si.on_wait = w[len(w) - K:]
                insts[i:i] = carriers
                i += len(carriers)
            i += 1
    for fn in nc.m.functions:
        for bb in fn.blocks:
            fix_block(bb)


def _bp_bcast_ap(bass, bp_d):
    a = bp_d[:]
    return bass.AP(tensor=a.tensor, offset=a.offset, ap=[[0, P]] + list(a.ap))


def build_nc(B_loc=B // N_CORES, chunk_tok=1024):
    import concourse.bass as bass
    import concourse.tile as tile
    from concourse import mybir
    from contextlib import ExitStack

    _patch_tile_drain(tile, mybir)
    _install_loud_cc_hook()

    F32 = mybir.dt.float32
    BF16 = mybir.dt.bfloat16
    F8 = mybir.dt.float8e4
    AF = mybir.ActivationFunctionType
    ALU = mybir.AluOpType
    DR = mybir.MatmulPerfMode.DoubleRow

    BT = B_loc * T
    chunk_tok = min(chunk_tok, BT)
    n_chunks = BT // chunk_tok
    assert n_chunks * chunk_tok == BT
    TT = chunk_tok // P     # 128-token tiles per chunk
    NB = chunk_tok // T     # batches per chunk

    W8SC = 1024.0           # fp8 weight pre-scale (2^10)
    EXPSC = 0.125 * (2.0 ** -20) if F8CC > 0 else 0.125
    # the bf16 contraction tail would accumulate unscaled q/k into the
    # 2^10-scaled fp8 PSUM; scale its weights too if ever enabled
    assert F8CC in (0, 3), "mixed-precision tail needs scaled bf16 weights"

    nc = bass.Bass()
    x_d = nc.declare_dram_parameter("x", [B_loc, T, C], F32, isOutput=False)
    wq_d = nc.declare_dram_parameter("Wq", [H, Dh, C], F32, isOutput=False)
    wk_d = nc.declare_dram_parameter("Wk", [H, Dh, C], F32, isOutput=False)
    wv_d = nc.declare_dram_parameter("Wv", [H, Dh, C], F32, isOutput=False)
    wp_d = nc.declare_dram_parameter("Wp", [C, HD], F32, isOutput=False)
    bp_d = nc.declare_dram_parameter("bp", [C], F32, isOutput=False)
    mk_d = nc.declare_dram_parameter("mask", [P, 2 * NHD * T], BF16, isOutput=False)
    out_d = nc.declare_dram_parameter("out", [B_loc, T, C], F32, isOutput=True)

    xf = x_d[:].flatten_outer_dims()      # [BT, C]
    of = out_d[:].flatten_outer_dims()    # [BT, C]

    with tile.TileContext(nc) as tc, ExitStack() as ctx:
        sing = ctx.enter_context(tc.tile_pool(name="sing", bufs=1))
        w4_p = ctx.enter_context(tc.tile_pool(name="w4", bufs=2 if F8CC == 3 else 4))
        wbf_p = ctx.enter_context(tc.tile_pool(name="wbfp", bufs=8))
        xbf_p = ctx.enter_context(tc.tile_pool(name="xbfp", bufs=10))
        xT_p = ctx.enter_context(tc.tile_pool(name="xTp", bufs=2))
        x8_p = ctx.enter_context(tc.tile_pool(name="x8p", bufs=2))
        qT_p = ctx.enter_context(tc.tile_pool(name="qTp", bufs=1))
        vsb_p = ctx.enter_context(tc.tile_pool(name="vsb", bufs=3))
        pex_p = ctx.enter_context(tc.tile_pool(name="pex", bufs=1))
        y_p = ctx.enter_context(tc.tile_pool(name="y", bufs=4))
        yt_p = ctx.enter_context(tc.tile_pool(name="yt", bufs=6))
        ostage = ctx.enter_context(tc.tile_pool(name="ostage", bufs=3))
        small = ctx.enter_context(tc.tile_pool(name="small", bufs=6))
        pp = ctx.enter_context(tc.tile_pool(name="pp", bufs=8, space="PSUM"))

        def ptile(pdim, shape, name, dt=None):
            # all PSUM tiles share one 1-bank slot class for max in-flight tiles
            t = pp.tile([P, 512], dt or F32, tag="ps", name=name)
            flat = t[:pdim, : int(np.prod(shape[1:]))]
            return flat.rearrange(
                "p (a b) -> p a b", a=shape[1]
            ) if len(shape) == 3 else flat

        # ---- weight loads first: these DMAs gate the first chunk ----
        # each W: 6 casting DMAs (f32 DRAM -> bf16 SBUF) + 6 XBAR transposes
        # wT4[p, r, b, s] = Wflat[r*128 + s, b*128 + p]
        wflats = {
            "wk": wk_d[:].flatten_outer_dims(),
            "wq": wq_d[:].flatten_outer_dims(),
            "wv": wv_d[:].flatten_outer_dims(),
            "wp": wp_d[:],
        }
        # ---- weight load + XBAR transpose (load on SWDGE, XBAR on scalar) ----
        # wT4[p, r, b, s] = Wflat[r*128 + s, b*128 + p]
        def prep_w4(wname, pool):
            wT4 = pool.tile([P, 6, NCC, P], BF16, name=f"{wname}T4", tag="w4")
            for r in range(6):
                wbf = wbf_p.tile([P, C], BF16, tag="wbf", name=f"{wname}_bf{r}")
                nc.gpsimd.dma_start(out=wbf, in_=wflats[wname][r * P:(r + 1) * P, :])
                nc.scalar.dma_start_transpose(out=wT4[:, r, :, :], in_=wbf)
            return wT4

        # fp8 copies of wk/wq (x W8SC), on DVE, sliced per m for pipelining
        def cast_w8(wT4, name):
            w8 = sing.tile([P, 6, NCC, P], F8, name=name)
            for m in range(6):
                nc.vector.tensor_scalar_mul(w8[:, m], wT4[:, m], W8SC)
            return w8

        # ---- x loads (casting DMAs on the SWDGE queue) ----
        def p0a(ci):
            tok0 = ci * chunk_tok
            xbs = []
            for it in range(TT):
                xb = xbf_p.tile([P, C], BF16, tag="xbf")
                nc.gpsimd.dma_start(out=xb, in_=xf[tok0 + it * P:tok0 + (it + 1) * P, :])
                xbs.append(xb)
            return xbs

        # priority order: first chunk's QK needs wk/wq + x; wv/wp trail
        wkT4 = prep_w4("wk", w4_p)
        wqT4 = prep_w4("wq", w4_p)
        wk8 = cast_w8(wkT4, "wk8")
        wq8 = cast_w8(wqT4, "wq8")
        xbs_cur = p0a(0)

        mask_sb = sing.tile([P, 2, NHD, T], BF16)
        nc.sync.dma_start(out=mask_sb, in_=mk_d[:].rearrange(
            "p (two a b) -> p two a b", two=2, a=NHD))

        # ktbd zeros must land before chunk 0's K evacuation
        ktbd = sing.tile([P, NHD, NB, P], BF16, name="ktbd")
        nc.vector.memset(ktbd, 0.0)

        # wv/wp reuse the wk/wq bf16 buffers once the fp8 casts are done
        wvT4 = prep_w4("wv", w4_p)
        wpT4 = prep_w4("wp", w4_p)
        bp_bc = sing.tile([P, C], F32)
        nc.gpsimd.dma_start(out=bp_bc, in_=_bp_bcast_ap(bass, bp_d))
        # rhs views: [p, cc, m, s] ordering for V and O projections
        wvT4r = wvT4.rearrange("p m c s -> p c m s")
        wpT4r = wpT4.rearrange("p r j c -> p j r c")

        # ---- persistent block-diagonal V operand ----
        vbd = sing.tile([P, NHD, NB, 2 * (Dh + 1)], BF16, name="vbd")
        nc.vector.memset(vbd, 0.0)
        nc.vector.memset(vbd[0:T, :, :, Dh:Dh + 1], 1.0)
        nc.vector.memset(vbd[T:P, :, :, 2 * Dh + 1:2 * Dh + 2], 1.0)

        # ---- per-chunk X pipeline: XBAR transpose + fp8 cast ----
        # x8 is cc-major [c-part, cc, tok] so each QK psum bank accumulates
        # in ONE start/stop group with 512-wide streams
        def p0b(xbs):
            xT = xT_p.tile([P, TT, NCC, P], BF16, tag="xT")
            x8 = x8_p.tile([P, NCC, chunk_tok], F8, tag="x8")
            for it in range(TT):
                nc.sync.dma_start_transpose(out=xT[:, it, :, :], in_=xbs[it])
            for it in range(TT):
                nc.vector.tensor_copy(
                    out=x8[:, :, it * P:(it + 1) * P], in_=xT[:, it])
            return xT, x8

        xT, x8 = p0b(xbs_cur)

        for ci in range(n_chunks):
            tok0 = ci * chunk_tok

            # next chunk X loads first: DMAs run during P1a, XBARs + fp8
            # casts (emitted below) land while sync/DVE queues are idle
            if ci + 1 < n_chunks:
                xbs_next = p0a(ci + 1)

            # ---- P1a: Q/K projections, fp8 DoubleRow ----
            qT = qT_p.tile([P, NHD, chunk_tok], BF16, tag="qT")
            nbsub = 512 // T
            for w8t, wT4t, dst in ((wk8, wkT4, "k"), (wq8, wqT4, "q")):
                for m in range(NHD):
                    for s in range(TT // 4):
                        psf = ptile(P, (P, 512), f"qk_{dst}{m}{s}")
                        for cc in range(F8CC):
                            nc.tensor.matmul(
                                psf, w8t[:, m, 2 * cc:2 * cc + 2, :],
                                x8[:, 2 * cc:2 * cc + 2, s * 512:(s + 1) * 512],
                                start=(cc == 0), stop=(cc == F8CC - 1),
                                perf_mode=DR)
                        if dst == "q":
                            nc.vector.tensor_copy(
                                out=qT[:, m, s * 512:(s + 1) * 512],
                                in_=psf)
                        else:
                            b0 = s * nbsub
                            nc.scalar.copy(
                                out=ktbd[0:T, m, b0:b0 + nbsub, 0:T],
                                in_=psf[0:T].rearrange(
                                    "p (nb t) -> p nb t", nb=nbsub))
                            nc.scalar.copy(
                                out=ktbd[T:P, m, b0:b0 + nbsub, T:P],
                                in_=psf[T:P].rearrange(
                                    "p (nb t) -> p nb t", nb=nbsub))

            if ci + 1 < n_chunks:
                xT_next, x8_next = p0b(xbs_next)

            # ---- P2a+P1b interleaved per tile: scores+exp+mask | V proj ----
            pex_all = pex_p.tile([P, NB, NHD, T], BF16, tag="pex")
            vbd_v = vbd.rearrange("p a (nb2 two) c -> p a nb2 two c", two=2)
            for it in range(TT):
                for half in range(2):
                    b = 2 * it + half
                    s_ps = ptile(P, (P, NHD, T), f"s_ps{b % 2}")
                    for m in range(NHD):
                        nc.tensor.matmul(
                            s_ps[:, m, :], ktbd[:, m, b, :],
                            qT[:, m, b * T:(b + 1) * T],
                            start=True, stop=True)
                    nc.scalar.activation(
                        out=pex_all[:, b], in_=s_ps, func=AF.Exp, scale=EXPSC)
                nc.vector.tensor_tensor(
                    pex_all[:, 2 * it:2 * it + 2], pex_all[:, 2 * it:2 * it + 2],
                    mask_sb, ALU.mult)
                # V projection for this tile
                psA = ptile(P, (P, 512), "v_psA")
                psB = ptile(P, (P, 256), "v_psB")
                for cc in range(NCC):
                    lhs = xT[:, it, cc, :]
                    nc.tensor.matmul(psA, lhs, wvT4r[:, cc, 0:4, :],
                                     start=(cc == 0), stop=(cc == NCC - 1))
                    nc.tensor.matmul(psB, lhs, wvT4r[:, cc, 4:6, :],
                                     start=(cc == 0), stop=(cc == NCC - 1))
                v_sb = vsb_p.tile([P, H, Dh], BF16, tag="v_sb")
                nc.scalar.copy(
                    out=v_sb[:, 0:8, :], in_=psA.rearrange("p (a b) -> p a b", a=8))
                nc.scalar.copy(
                    out=v_sb[:, 8:12, :], in_=psB.rearrange("p (a b) -> p a b", a=4))
                v_sb2 = v_sb.rearrange("p (h two) c -> p h two c", two=2)
                for par in range(2):
                    nc.gpsimd.dma_start(
                        out=vbd_v[0:T, :, it, par, 0:Dh],
                        in_=v_sb2[par * T:(par + 1) * T, :, 0, :])
                    nc.gpsimd.dma_start(
                        out=vbd_v[T:P, :, it, par, Dh + 1:2 * Dh + 1],
                        in_=v_sb2[par * T:(par + 1) * T, :, 1, :])

            # ---- P2b/P3: AV + normalize + XBAR-Y + output projection ----
            ybs = []
            ytiles = []
            def yt_tr(yb):
                ytile = yt_p.tile([P, NHD, P], BF16, tag="ytile")
                nc.sync.dma_start_transpose(out=ytile, in_=yb)
                ytiles.append(ytile)
            def oproj_emit(it):
                ytile = ytiles[it]
                oA = ptile(P, (P, 512), "o_psA")
                oB = ptile(P, (P, 256), "o_psB")
                for j in range(NHD):
                    lhs = ytile[:, j, :]
                    nc.tensor.matmul(oA, lhs, wpT4r[:, j, 0:4, :],
                                     start=(j == 0), stop=(j == NHD - 1))
                    nc.tensor.matmul(oB, lhs, wpT4r[:, j, 4:6, :],
                                     start=(j == 0), stop=(j == NHD - 1))
                osb = ostage.tile([P, C], F32, tag="osb")
                nc.vector.tensor_tensor(osb[:, 0:512], oA, bp_bc[:, 0:512], ALU.add)
                nc.vector.tensor_tensor(osb[:, 512:768], oB, bp_bc[:, 512:768], ALU.add)
                row0 = tok0 + it * P
                nc.sync.dma_start(out=of[row0:row0 + P, :], in_=osb)

            for it in range(TT):
                yb = y_p.tile([P, HD], BF16, tag="yb")
                ybs.append(yb)
                y_ps = [ptile(P, (P, 3, 2 * (Dh + 1)), f"y_ps{h2}") for h2 in range(2)]
                for m in range(NHD):
                    for half in range(2):
                        b = it * 2 + half
                        prow = half * T
                        nc.tensor.matmul(
                            y_ps[m // 3][prow:prow + T, m % 3, :],
                            pex_all[:, b, m, :],
                            vbd[:, m, b, :],
                            start=True, stop=True)
                for h2 in range(2):
                    y_v = y_ps[h2].rearrange("p a (two c) -> p a two c", c=Dh + 1)
                    rec = small.tile([P, 3, 2, 1], F32, tag="rec", name="rec")
                    nc.vector.reciprocal(out=rec, in_=y_v[:, :, :, Dh:Dh + 1])
                    nc.vector.tensor_tensor(
                        yb[:, h2 * 384:(h2 + 1) * 384]
                            .rearrange("p (a two b) -> p a two b", a=3, two=2),
                        y_v[:, :, :, 0:Dh],
                        rec.to_broadcast([P, 3, 2, Dh]),
                        ALU.mult)
                if it >= 2:
                    yt_tr(ybs[it - 2])
                if it >= 3:
                    oproj_emit(it - 3)
            yt_tr(ybs[TT - 2])
            oproj_emit(TT - 3)
            yt_tr(ybs[TT - 1])
            oproj_emit(TT - 2)
            oproj_emit(TT - 1)
            if ci + 1 < n_chunks:
                xT, x8 = xT_next, x8_next

    _split_multi_waits(nc, mybir)
    return nc


def _get_program(B_loc, chunk_tok):
    key = (B_loc, chunk_tok)
    if key not in _cache:
        _cache[key] = build_nc(B_loc, chunk_tok)
    return _cache[key]


def make_const_inputs():
    import ml_dtypes
    # mask[s, t] = 1 if s <= t (causal, scoresT layout)
    m = np.tril(np.ones((T, T), dtype=np.float32)).T
    m2 = np.vstack([m, m])   # replicated for both partition-halves
    mask = np.tile(m2, (1, 2 * NHD)).astype(ml_dtypes.bfloat16)  # [P, 2*NHD*T]
    return mask


def prepare(x, Wq, Wk, Wv, Wp, bp, chunk_tok=1024):
    x = np.ascontiguousarray(x, dtype=np.float32)
    B_loc = B // N_CORES
    mask = make_const_inputs()
    nc = _get_program(B_loc, chunk_tok)
    in_maps = []
    for c in range(N_CORES):
        in_maps.append({
            "x": x[c * B_loc:(c + 1) * B_loc],
            "Wq": np.ascontiguousarray(Wq, dtype=np.float32),
            "Wk": np.ascontiguousarray(Wk, dtype=np.float32),
            "Wv": np.ascontiguousarray(Wv, dtype=np.float32),
            "Wp": np.ascontiguousarray(Wp, dtype=np.float32),
            "bp": np.ascontiguousarray(bp, dtype=np.float32),
            "mask": mask,
        })
    return nc, in_maps


def kernel(x, Wq, Wk, Wv, Wp, bp):
    from concourse import bass_utils

    nc, in_maps = prepare(x, Wq, Wk, Wv, Wp, bp)
    res = bass_utils.run_bass_kernel_spmd(nc, in_maps, list(range(N_CORES)))
    return np.concatenate([res.results[c]["out"] for c in range(N_CORES)], axis=0)


# revision 22
# speedup vs baseline: 1.0156x; 1.0156x over previous
"""Multi-head causal attention (B=1024, T=64, C=768, H=12, D=64) on 8 TRN2
NeuronCores, data-parallel over the batch dimension (128 batches/core).

v2 dataflow per core (1024-token chunks, TT=8 128-token tiles, NB=16 batches):
  - All transposes ride the XBAR DMA engine (dma_start_transpose, 14ns per
    16x128 tile): X tiles, Y tiles and all four weight preps. The PE runs
    only "real" matmuls.
  - Q/K projections run in fp8(e4m3) DoubleRow perf mode (2 contraction
    rows/partition/cycle): W is pre-scaled by 2^10 into fp8, X is cast
    straight to fp8 (validated: final rel err ~1.6e-2 < 2e-2 gate); the
    2^-20 descale is folded into the softmax exp scale. V / output
    projections stay bf16.
  - scores+exp and the V projection interleave per tile so the Scalar exp
    drains while the PE streams V; AV / normalize / XBAR-Y / output
    projection software-pipeline per tile as before.
  - Evacuations are spread across engines: q+mask+normalize+recip+x-fp8
    casts on DVE, k+v_sb+exp on Scalar, out-bias add on GpSimd, XBARs on
    the SP/Activation HWDGE queues.
"""

import numpy as np

P = 128
B, T, C, H, Dh = 1024, 64, 768, 12, 64
HD = H * Dh            # 768
NCC = C // P           # 6 contraction chunks
NHD = HD // P          # 6 hd chunks
N_CORES = 8
F8CC = 3               # how many 256-channel pairs of the QK contraction run fp8
                       # (3 = all 768 channels; 2 = 512 fp8 + 256 bf16 fallback)

_cache = {}


def _patch_tile_drain(tile, mybir):
    """walrus CTRL (Drain) ops in this toolchain accept only 1 sem-wait;
    spread the TileContext exit-drain's waits across preceding SP nops."""
    from concourse.vector_clock import ScopedClock

    if getattr(tile.TileContext, "_drain_patched", False):
        return

    def _drain_and_barrier(self, tick_clock, wait_clock):
        nc = self.nc
        drain_inst = nc.sync.drain()
        wait_clock.add_sem_waits(
            drain_inst.ins, ScopedClock({None: tick_clock.global_clock})
        )
        waits = list(drain_inst.ins.sync_info.on_wait)
        if len(waits) > 1:
            drain_inst.ins.sync_info.on_wait = waits[-1:]
            cur_bb = nc.cur_bb.bb
            idx = cur_bb.instructions.index(drain_inst.ins)
            extra = []
            for w in waits[:-1]:
                nop = mybir.InstNoOp(name=f"I-{nc.next_id()}", ins=[], outs=[])
                nop.engine = drain_inst.ins.engine
                nop.sync_info = mybir.SyncInfo(on_wait=[w], on_update=[])
                nc.register_instruction(nop)
                extra.append(nop)
            cur_bb.instructions[idx:idx] = extra
        nc.all_engine_barrier()
        assert self.sems is not None
        popped = nc._tile_sem_poison_stack.pop()
        assert popped is self._sem_poison
        nc.clear_and_free_semaphores(list(self.sems.allocated().values()))
        nc.all_engine_barrier()

    tile.TileContext._drain_and_barrier = _drain_and_barrier
    tile.TileContext._drain_patched = True


def _install_loud_cc_hook():
    """Surface real exceptions from the neuronx_cc hook (C wrapper eats them)."""
    from concourse import bass2jax as _b2j
    if getattr(_b2j, "_loud_hook_installed", False):
        return
    _orig = _b2j.neuronx_cc_hook
    def _loud(*a, **k):
        try:
            return _orig(*a, **k)
        except BaseException:
            import traceback
            traceback.print_exc()
            raise
    _b2j.neuronx_cc_hook = _loud
    _b2j._loud_hook_installed = True


def _split_multi_waits(nc, mybir, K=1):
    """This walrus build supports only one sem-wait per instruction: move
    excess waits onto same-engine NOPs inserted directly before the owner."""
    def fix_block(bb):
        insts = bb.instructions
        i = 0
        while i < len(insts):
            ins = insts[i]
            si = ins.sync_info
            w = list(si.on_wait) if si is not None and si.on_wait else []
            if len(w) > K:
                carriers = []
                for j in range(0, len(w) - K, K):
                    nop = mybir.InstNoOp(name=f"I-{nc.next_id()}", ins=[], outs=[])
                    nop.engine = ins.engine
                    nop.sync_info = mybir.SyncInfo(on_wait=w[j:j + K], on_update=[])
                    nc.register_instruction(nop)
                    carriers.append(nop)
                si.on_wait = w[len(w) - K:]
                insts[i:i] = carriers
                i += len(carriers)
            i += 1
    for fn in nc.m.functions:
        for bb in fn.blocks:
            fix_block(bb)


def _bp_bcast_ap(bass, bp_d):
    a = bp_d[:]
    return bass.AP(tensor=a.tensor, offset=a.offset, ap=[[0, P]] + list(a.ap))


def build_nc(B_loc=B // N_CORES, chunk_tok=1024):
    import concourse.bass as bass
    import concourse.tile as tile
    from concourse import mybir
    from contextlib import ExitStack

    _patch_tile_drain(tile, mybir)
    _install_loud_cc_hook()

    F32 = mybir.dt.float32
    BF16 = mybir.dt.bfloat16
    F8 = mybir.dt.float8e4
    AF = mybir.ActivationFunctionType
    ALU = mybir.AluOpType
    DR = mybir.MatmulPerfMode.DoubleRow

    BT = B_loc * T
    chunk_tok = min(chunk_tok, BT)
    n_chunks = BT // chunk_tok
    assert n_chunks * chunk_tok == BT
    TT = chunk_tok // P     # 128-token tiles per chunk
    NB = chunk_tok // T     # batches per chunk

    W8SC = 1024.0           # fp8 weight pre-scale (2^10)
    EXPSC = 0.125 * (2.0 ** -20) if F8CC > 0 else 0.125
    # the bf16 contraction tail would accumulate unscaled q/k into the
    # 2^10-scaled fp8 PSUM; scale its weights too if ever enabled
    assert F8CC in (0, 3), "mixed-precision tail needs scaled bf16 weights"

    nc = bass.Bass()
    x_d = nc.declare_dram_parameter("x", [B_loc, T, C], F32, isOutput=False)
    wq_d = nc.declare_dram_parameter("Wq", [H, Dh, C], F32, isOutput=False)
    wk_d = nc.declare_dram_parameter("Wk", [H, Dh, C], F32, isOutput=False)
    wv_d = nc.declare_dram_parameter("Wv", [H, Dh, C], F32, isOutput=False)
    wp_d = nc.declare_dram_parameter("Wp", [C, HD], F32, isOutput=False)
    bp_d = nc.declare_dram_parameter("bp", [C], F32, isOutput=False)
    mk_d = nc.declare_dram_parameter("mask", [P, 2 * NHD * T], BF16, isOutput=False)
    out_d = nc.declare_dram_parameter("out", [B_loc, T, C], F32, isOutput=True)

    xf = x_d[:].flatten_outer_dims()      # [BT, C]
    of = out_d[:].flatten_outer_dims()    # [BT, C]

    with tile.TileContext(nc) as tc, ExitStack() as ctx:
        sing = ctx.enter_context(tc.tile_pool(name="sing", bufs=1))
        w4_p = ctx.enter_context(tc.tile_pool(name="w4", bufs=2 if F8CC == 3 else 4))
        wbf_p = ctx.enter_context(tc.tile_pool(name="wbfp", bufs=8))
        xbf_p = ctx.enter_context(tc.tile_pool(name="xbfp", bufs=10))
        xT_p = ctx.enter_context(tc.tile_pool(name="xTp", bufs=2))
        x8_p = ctx.enter_context(tc.tile_pool(name="x8p", bufs=2))
        qT_p = ctx.enter_context(tc.tile_pool(name="qTp", bufs=1))
        vsb_p = ctx.enter_context(tc.tile_pool(name="vsb", bufs=3))
        pex_p = ctx.enter_context(tc.tile_pool(name="pex", bufs=1))
        y_p = ctx.enter_context(tc.tile_pool(name="y", bufs=4))
        yt_p = ctx.enter_context(tc.tile_pool(name="yt", bufs=6))
        ostage = ctx.enter_context(tc.tile_pool(name="ostage", bufs=3))
        small = ctx.enter_context(tc.tile_pool(name="small", bufs=6))
        pp = ctx.enter_context(tc.tile_pool(name="pp", bufs=8, space="PSUM"))

        def ptile(pdim, shape, name, dt=None):
            # all PSUM tiles share one 1-bank slot class for max in-flight tiles
            t = pp.tile([P, 512], dt or F32, tag="ps", name=name)
            flat = t[:pdim, : int(np.prod(shape[1:]))]
            return flat.rearrange(
                "p (a b) -> p a b", a=shape[1]
            ) if len(shape) == 3 else flat

        # ---- weight loads first: these DMAs gate the first chunk ----
        # each W: 6 casting DMAs (f32 DRAM -> bf16 SBUF) + 6 XBAR transposes
        # wT4[p, r, b, s] = Wflat[r*128 + s, b*128 + p]
        wflats = {
            "wk": wk_d[:].flatten_outer_dims(),
            "wq": wq_d[:].flatten_outer_dims(),
            "wv": wv_d[:].flatten_outer_dims(),
            "wp": wp_d[:],
        }
        # ---- weight load + XBAR transpose (load on SWDGE, XBAR on scalar) ----
        # wT4[p, r, b, s] = Wflat[r*128 + s, b*128 + p]
        def prep_w4(wname, pool):
            wT4 = pool.tile([P, 6, NCC, P], BF16, name=f"{wname}T4", tag="w4")
            for r in range(6):
                wbf = wbf_p.tile([P, C], BF16, tag="wbf", name=f"{wname}_bf{r}")
                nc.gpsimd.dma_start(out=wbf, in_=wflats[wname][r * P:(r + 1) * P, :])
                nc.scalar.dma_start_transpose(out=wT4[:, r, :, :], in_=wbf)
            return wT4

        # fp8 copies of wk/wq (x W8SC), on DVE, sliced per m for pipelining
        def cast_w8(wT4, name):
            w8 = sing.tile([P, 6, NCC, P], F8, name=name)
            for m in range(6):
                nc.vector.tensor_scalar_mul(w8[:, m], wT4[:, m], W8SC)
            return w8

        # ---- x loads (casting DMAs on the SWDGE queue) ----
        def p0a(ci):
            tok0 = ci * chunk_tok
            xbs = []
            for it in range(TT):
                xb = xbf_p.tile([P, C], BF16, tag="xbf")
                nc.gpsimd.dma_start(out=xb, in_=xf[tok0 + it * P:tok0 + (it + 1) * P, :])
                xbs.append(xb)
            return xbs

        # priority order: first chunk's QK needs wk/wq + x; wv/wp trail
        wkT4 = prep_w4("wk", w4_p)
        wqT4 = prep_w4("wq", w4_p)
        wk8 = cast_w8(wkT4, "wk8")
        wq8 = cast_w8(wqT4, "wq8")
        xbs_cur = p0a(0)

        mask_sb = sing.tile([P, 2, NHD, T], BF16)
        nc.sync.dma_start(out=mask_sb, in_=mk_d[:].rearrange(
            "p (two a b) -> p two a b", two=2, a=NHD))

        # ktbd zeros must land before chunk 0's K evacuation
        ktbd = sing.tile([P, NHD, NB, P], BF16, name="ktbd")
        nc.vector.memset(ktbd, 0.0)

        # wv/wp reuse the wk/wq bf16 buffers once the fp8 casts are done
        wvT4 = prep_w4("wv", w4_p)
        wpT4 = prep_w4("wp", w4_p)
        bp_bc = sing.tile([P, C], F32)
        nc.gpsimd.dma_start(out=bp_bc, in_=_bp_bcast_ap(bass, bp_d))
        # rhs views: [p, cc, m, s] ordering for V and O projections
        wvT4r = wvT4.rearrange("p m c s -> p c m s")
        wpT4r = wpT4.rearrange("p r j c -> p j r c")

        # ---- persistent block-diagonal V operand ----
        vbd = sing.tile([P, NHD, NB, 2 * (Dh + 1)], BF16, name="vbd")
        nc.vector.memset(vbd, 0.0)
        nc.vector.memset(vbd[0:T, :, :, Dh:Dh + 1], 1.0)
        nc.vector.memset(vbd[T:P, :, :, 2 * Dh + 1:2 * Dh + 2], 1.0)

        # ---- per-chunk X pipeline: XBAR transpose + fp8 cast ----
        # x8 is cc-major [c-part, cc, tok] so each QK psum bank accumulates
        # in ONE start/stop group with 512-wide streams
        def p0b(xbs):
            xT = xT_p.tile([P, TT, NCC, P], BF16, tag="xT")
            x8 = x8_p.tile([P, NCC, chunk_tok], F8, tag="x8")
            for it in range(TT):
                nc.scalar.dma_start_transpose(out=xT[:, it, :, :], in_=xbs[it])
            for it in range(TT):
                nc.vector.tensor_copy(
                    out=x8[:, :, it * P:(it + 1) * P], in_=xT[:, it])
            return xT, x8

        xT, x8 = p0b(xbs_cur)

        for ci in range(n_chunks):
            tok0 = ci * chunk_tok

            # next chunk X loads first: DMAs run during P1a, XBARs + fp8
            # casts (emitted below) land while sync/DVE queues are idle
            if ci + 1 < n_chunks:
                xbs_next = p0a(ci + 1)

            # ---- P1a: Q/K projections, fp8 DoubleRow ----
            qT = qT_p.tile([P, NHD, chunk_tok], BF16, tag="qT")
            nbsub = 512 // T
            for w8t, wT4t, dst in ((wk8, wkT4, "k"), (wq8, wqT4, "q")):
                for m in range(NHD):
                    for s in range(TT // 4):
                        psf = ptile(P, (P, 512), f"qk_{dst}{m}{s}")
                        for cc in range(F8CC):
                            nc.tensor.matmul(
                                psf, w8t[:, m, 2 * cc:2 * cc + 2, :],
                                x8[:, 2 * cc:2 * cc + 2, s * 512:(s + 1) * 512],
                                start=(cc == 0), stop=(cc == F8CC - 1),
                                perf_mode=DR)
                        if dst == "q":
                            nc.vector.tensor_copy(
                                out=qT[:, m, s * 512:(s + 1) * 512],
                                in_=psf)
                        else:
                            b0 = s * nbsub
                            nc.scalar.copy(
                                out=ktbd[0:T, m, b0:b0 + nbsub, 0:T],
                                in_=psf[0:T].rearrange(
                                    "p (nb t) -> p nb t", nb=nbsub))
                            nc.scalar.copy(
                                out=ktbd[T:P, m, b0:b0 + nbsub, T:P],
                                in_=psf[T:P].rearrange(
                                    "p (nb t) -> p nb t", nb=nbsub))

            if ci + 1 < n_chunks:
                xT_next, x8_next = p0b(xbs_next)

            # ---- P2a+P1b interleaved per tile: scores+exp+mask | V proj ----
            pex_all = pex_p.tile([P, NB, NHD, T], BF16, tag="pex")
            vbd_v = vbd.rearrange("p a (nb2 two) c -> p a nb2 two c", two=2)
            for it in range(TT):
                for half in range(2):
                    b = 2 * it + half
                    s_ps = ptile(P, (P, NHD, T), f"s_ps{b % 2}")
                    for m in range(NHD):
                        nc.tensor.matmul(
                            s_ps[:, m, :], ktbd[:, m, b, :],
                            qT[:, m, b * T:(b + 1) * T],
                            start=True, stop=True)
                    nc.scalar.activation(
                        out=pex_all[:, b], in_=s_ps, func=AF.Exp, scale=EXPSC)
                nc.vector.tensor_tensor(
                    pex_all[:, 2 * it:2 * it + 2], pex_all[:, 2 * it:2 * it + 2],
                    mask_sb, ALU.mult)
                # V projection for this tile
                psA = ptile(P, (P, 512), "v_psA")
                psB = ptile(P, (P, 256), "v_psB")
                for cc in range(NCC):
                    lhs = xT[:, it, cc, :]
                    nc.tensor.matmul(psA, lhs, wvT4r[:, cc, 0:4, :],
                                     start=(cc == 0), stop=(cc == NCC - 1))
                    nc.tensor.matmul(psB, lhs, wvT4r[:, cc, 4:6, :],
                                     start=(cc == 0), stop=(cc == NCC - 1))
                v_sb = vsb_p.tile([P, H, Dh], BF16, tag="v_sb")
                nc.scalar.copy(
                    out=v_sb[:, 0:8, :], in_=psA.rearrange("p (a b) -> p a b", a=8))
                nc.scalar.copy(
                    out=v_sb[:, 8:12, :], in_=psB.rearrange("p (a b) -> p a b", a=4))
                v_sb2 = v_sb.rearrange("p (h two) c -> p h two c", two=2)
                for par in range(2):
                    nc.sync.dma_start(
                        out=vbd_v[0:T, :, it, par, 0:Dh],
                        in_=v_sb2[par * T:(par + 1) * T, :, 0, :])
                    nc.sync.dma_start(
                        out=vbd_v[T:P, :, it, par, Dh + 1:2 * Dh + 1],
                        in_=v_sb2[par * T:(par + 1) * T, :, 1, :])

            # ---- P2b/P3: AV + normalize + XBAR-Y + output projection ----
            ybs = []
            ytiles = []
            def yt_tr(yb):
                ytile = yt_p.tile([P, NHD, P], BF16, tag="ytile")
                nc.sync.dma_start_transpose(out=ytile, in_=yb)
                ytiles.append(ytile)
            def oproj_emit(it):
                ytile = ytiles[it]
                oA = ptile(P, (P, 512), "o_psA")
                oB = ptile(P, (P, 256), "o_psB")
                for j in range(NHD):
                    lhs = ytile[:, j, :]
                    nc.tensor.matmul(oA, lhs, wpT4r[:, j, 0:4, :],
                                     start=(j == 0), stop=(j == NHD - 1))
                    nc.tensor.matmul(oB, lhs, wpT4r[:, j, 4:6, :],
                                     start=(j == 0), stop=(j == NHD - 1))
                osb = ostage.tile([P, C], F32, tag="osb")
                nc.vector.tensor_tensor(osb[:, 0:512], oA, bp_bc[:, 0:512], ALU.add)
                nc.vector.tensor_tensor(osb[:, 512:768], oB, bp_bc[:, 512:768], ALU.add)
                row0 = tok0 + it * P
                nc.gpsimd.dma_start(out=of[row0:row0 + P, :], in_=osb)

            for it in range(TT):
                yb = y_p.tile([P, HD], BF16, tag="yb")
                ybs.append(yb)
                y_ps = [ptile(P, (P, 3, 2 * (Dh + 1)), f"y_ps{h2}") for h2 in range(2)]
                for m in range(NHD):
                    for half in range(2):
                        b = it * 2 + half
                        prow = half * T
                        nc.tensor.matmul(
                            y_ps[m // 3][prow:prow + T, m % 3, :],
                            pex_all[:, b, m, :],
                            vbd[:, m, b, :],
                            start=True, stop=True)
                for h2 in range(2):
                    y_v = y_ps[h2].rearrange("p a (two c) -> p a two c", c=Dh + 1)
                    rec = small.tile([P, 3, 2, 1], F32, tag="rec", name="rec")
                    nc.vector.reciprocal(out=rec, in_=y_v[:, :, :, Dh:Dh + 1])
                    nc.vector.tensor_tensor(
                        yb[:, h2 * 384:(h2 + 1) * 384]
                            .rearrange("p (a two b) -> p a two b", a=3, two=2),
                        y_v[:, :, :, 0:Dh],
                        rec.to_broadcast([P, 3, 2, Dh]),
                        ALU.mult)
                if it >= 2:
                    yt_tr(ybs[it - 2])
                if it >= 3:
                    oproj_emit(it - 3)
            yt_tr(ybs[TT - 2])
            oproj_emit(TT - 3)
            yt_tr(ybs[TT - 1])
            oproj_emit(TT - 2)
            oproj_emit(TT - 1)
            if ci + 1 < n_chunks:
                xT, x8 = xT_next, x8_next

    _split_multi_waits(nc, mybir)
    return nc


def _get_program(B_loc, chunk_tok):
    key = (B_loc, chunk_tok)
    if key not in _cache:
        _cache[key] = build_nc(B_loc, chunk_tok)
    return _cache[key]


def make_const_inputs():
    import ml_dtypes
    # mask[s, t] = 1 if s <= t (causal, scoresT layout)
    m = np.tril(np.ones((T, T), dtype=np.float32)).T
    m2 = np.vstack([m, m])   # replicated for both partition-halves
    mask = np.tile(m2, (1, 2 * NHD)).astype(ml_dtypes.bfloat16)  # [P, 2*NHD*T]
    return mask


def prepare(x, Wq, Wk, Wv, Wp, bp, chunk_tok=1024):
    x = np.ascontiguousarray(x, dtype=np.float32)
    B_loc = B // N_CORES
    mask = make_const_inputs()
    nc = _get_program(B_loc, chunk_tok)
    in_maps = []
    for c in range(N_CORES):
        in_maps.append({
            "x": x[c * B_loc:(c + 1) * B_loc],
            "Wq": np.ascontiguousarray(Wq, dtype=np.float32),
            "Wk": np.ascontiguousarray(Wk, dtype=np.float32),
            "Wv": np.ascontiguousarray(Wv, dtype=np.float32),
            "Wp": np.ascontiguousarray(Wp, dtype=np.float32),
            "bp": np.ascontiguousarray(bp, dtype=np.float32),
            "mask": mask,
        })
    return nc, in_maps


def kernel(x, Wq, Wk, Wv, Wp, bp):
    from concourse import bass_utils

    nc, in_maps = prepare(x, Wq, Wk, Wv, Wp, bp)
    res = bass_utils.run_bass_kernel_spmd(nc, in_maps, list(range(N_CORES)))
    return np.concatenate([res.results[c]["out"] for c in range(N_CORES)], axis=0)


# revision 23
# speedup vs baseline: 1.0466x; 1.0306x over previous
"""Multi-head causal attention (B=1024, T=64, C=768, H=12, D=64) on 8 TRN2
NeuronCores, data-parallel over the batch dimension (128 batches/core).

v2 dataflow per core (1024-token chunks, TT=8 128-token tiles, NB=16 batches):
  - All transposes ride the XBAR DMA engine (dma_start_transpose, 14ns per
    16x128 tile): X tiles, Y tiles and all four weight preps. The PE runs
    only "real" matmuls.
  - Q/K projections run in fp8(e4m3) DoubleRow perf mode (2 contraction
    rows/partition/cycle): W is pre-scaled by 2^10 into fp8, X is cast
    straight to fp8 (validated: final rel err ~1.6e-2 < 2e-2 gate); the
    2^-20 descale is folded into the softmax exp scale. V / output
    projections stay bf16.
  - scores+exp and the V projection interleave per tile so the Scalar exp
    drains while the PE streams V; AV / normalize / XBAR-Y / output
    projection software-pipeline per tile as before.
  - Evacuations are spread across engines: q+mask+normalize+recip+x-fp8
    casts on DVE, k+v_sb+exp on Scalar, out-bias add on GpSimd, XBARs on
    the SP/Activation HWDGE queues.
"""

import numpy as np

P = 128
B, T, C, H, Dh = 1024, 64, 768, 12, 64
HD = H * Dh            # 768
NCC = C // P           # 6 contraction chunks
NHD = HD // P          # 6 hd chunks
N_CORES = 8
F8CC = 3               # how many 256-channel pairs of the QK contraction run fp8
                       # (3 = all 768 channels; 2 = 512 fp8 + 256 bf16 fallback)

_cache = {}


def _patch_tile_drain(tile, mybir):
    """walrus CTRL (Drain) ops in this toolchain accept only 1 sem-wait;
    spread the TileContext exit-drain's waits across preceding SP nops."""
    from concourse.vector_clock import ScopedClock

    if getattr(tile.TileContext, "_drain_patched", False):
        return

    def _drain_and_barrier(self, tick_clock, wait_clock):
        nc = self.nc
        drain_inst = nc.sync.drain()
        wait_clock.add_sem_waits(
            drain_inst.ins, ScopedClock({None: tick_clock.global_clock})
        )
        waits = list(drain_inst.ins.sync_info.on_wait)
        if len(waits) > 1:
            drain_inst.ins.sync_info.on_wait = waits[-1:]
            cur_bb = nc.cur_bb.bb
            idx = cur_bb.instructions.index(drain_inst.ins)
            extra = []
            for w in waits[:-1]:
                nop = mybir.InstNoOp(name=f"I-{nc.next_id()}", ins=[], outs=[])
                nop.engine = drain_inst.ins.engine
                nop.sync_info = mybir.SyncInfo(on_wait=[w], on_update=[])
                nc.register_instruction(nop)
                extra.append(nop)
            cur_bb.instructions[idx:idx] = extra
        nc.all_engine_barrier()
        assert self.sems is not None
        popped = nc._tile_sem_poison_stack.pop()
        assert popped is self._sem_poison
        nc.clear_and_free_semaphores(list(self.sems.allocated().values()))
        nc.all_engine_barrier()

    tile.TileContext._drain_and_barrier = _drain_and_barrier
    tile.TileContext._drain_patched = True


def _install_loud_cc_hook():
    """Surface real exceptions from the neuronx_cc hook (C wrapper eats them)."""
    from concourse import bass2jax as _b2j
    if getattr(_b2j, "_loud_hook_installed", False):
        return
    _orig = _b2j.neuronx_cc_hook
    def _loud(*a, **k):
        try:
            return _orig(*a, **k)
        except BaseException:
            import traceback
            traceback.print_exc()
            raise
    _b2j.neuronx_cc_hook = _loud
    _b2j._loud_hook_installed = True


def _split_multi_waits(nc, mybir, K=1):
    """This walrus build supports only one sem-wait per instruction: move
    excess waits onto same-engine NOPs inserted directly before the owner."""
    def fix_block(bb):
        insts = bb.instructions
        i = 0
        while i < len(insts):
            ins = insts[i]
            si = ins.sync_info
            w = list(si.on_wait) if si is not None and si.on_wait else []
            if len(w) > K:
                carriers = []
                for j in range(0, len(w) - K, K):
                    nop = mybir.InstNoOp(name=f"I-{nc.next_id()}", ins=[], outs=[])
                    nop.engine = ins.engine
                    nop.sync_info = mybir.SyncInfo(on_wait=w[j:j + K], on_update=[])
                    nc.register_instruction(nop)
                    carriers.append(nop)
                si.on_wait = w[len(w) - K:]
                insts[i:i] = carriers
                i += len(carriers)
            i += 1
    for fn in nc.m.functions:
        for bb in fn.blocks:
            fix_block(bb)


def _bp_bcast_ap(bass, bp_d):
    a = bp_d[:]
    return bass.AP(tensor=a.tensor, offset=a.offset, ap=[[0, P]] + list(a.ap))


def build_nc(B_loc=B // N_CORES, chunk_tok=1024):
    import concourse.bass as bass
    import concourse.tile as tile
    from concourse import mybir
    from contextlib import ExitStack

    _patch_tile_drain(tile, mybir)
    _install_loud_cc_hook()

    F32 = mybir.dt.float32
    BF16 = mybir.dt.bfloat16
    F8 = mybir.dt.float8e4
    AF = mybir.ActivationFunctionType
    ALU = mybir.AluOpType
    DR = mybir.MatmulPerfMode.DoubleRow

    BT = B_loc * T
    chunk_tok = min(chunk_tok, BT)
    n_chunks = BT // chunk_tok
    assert n_chunks * chunk_tok == BT
    TT = chunk_tok // P     # 128-token tiles per chunk
    NB = chunk_tok // T     # batches per chunk

    W8SC = 1024.0           # fp8 weight pre-scale (2^10)
    EXPSC = 0.125 * (2.0 ** -20) if F8CC > 0 else 0.125
    # the bf16 contraction tail would accumulate unscaled q/k into the
    # 2^10-scaled fp8 PSUM; scale its weights too if ever enabled
    assert F8CC in (0, 3), "mixed-precision tail needs scaled bf16 weights"

    nc = bass.Bass()
    x_d = nc.declare_dram_parameter("x", [B_loc, T, C], F32, isOutput=False)
    wq_d = nc.declare_dram_parameter("Wq", [H, Dh, C], F32, isOutput=False)
    wk_d = nc.declare_dram_parameter("Wk", [H, Dh, C], F32, isOutput=False)
    wv_d = nc.declare_dram_parameter("Wv", [H, Dh, C], F32, isOutput=False)
    wp_d = nc.declare_dram_parameter("Wp", [C, HD], F32, isOutput=False)
    bp_d = nc.declare_dram_parameter("bp", [C], F32, isOutput=False)
    mk_d = nc.declare_dram_parameter("mask", [P, 2 * NHD * T], BF16, isOutput=False)
    out_d = nc.declare_dram_parameter("out", [B_loc, T, C], F32, isOutput=True)

    xf = x_d[:].flatten_outer_dims()      # [BT, C]
    of = out_d[:].flatten_outer_dims()    # [BT, C]

    with tile.TileContext(nc) as tc, ExitStack() as ctx:
        sing = ctx.enter_context(tc.tile_pool(name="sing", bufs=1))
        w4_p = ctx.enter_context(tc.tile_pool(name="w4", bufs=2 if F8CC == 3 else 4))
        wbf_p = ctx.enter_context(tc.tile_pool(name="wbfp", bufs=8))
        xbf_p = ctx.enter_context(tc.tile_pool(name="xbfp", bufs=10))
        xT_p = ctx.enter_context(tc.tile_pool(name="xTp", bufs=2))
        x8_p = ctx.enter_context(tc.tile_pool(name="x8p", bufs=2))
        qT_p = ctx.enter_context(tc.tile_pool(name="qTp", bufs=1))
        vsb_p = ctx.enter_context(tc.tile_pool(name="vsb", bufs=3))
        pex_p = ctx.enter_context(tc.tile_pool(name="pex", bufs=1))
        y_p = ctx.enter_context(tc.tile_pool(name="y", bufs=4))
        yt_p = ctx.enter_context(tc.tile_pool(name="yt", bufs=6))
        ostage = ctx.enter_context(tc.tile_pool(name="ostage", bufs=3))
        small = ctx.enter_context(tc.tile_pool(name="small", bufs=6))
        pp = ctx.enter_context(tc.tile_pool(name="pp", bufs=8, space="PSUM"))

        def ptile(pdim, shape, name, dt=None):
            # all PSUM tiles share one 1-bank slot class for max in-flight tiles
            t = pp.tile([P, 512], dt or F32, tag="ps", name=name)
            flat = t[:pdim, : int(np.prod(shape[1:]))]
            return flat.rearrange(
                "p (a b) -> p a b", a=shape[1]
            ) if len(shape) == 3 else flat

        # ---- weight loads first: these DMAs gate the first chunk ----
        # each W: 6 casting DMAs (f32 DRAM -> bf16 SBUF) + 6 XBAR transposes
        # wT4[p, r, b, s] = Wflat[r*128 + s, b*128 + p]
        wflats = {
            "wk": wk_d[:].flatten_outer_dims(),
            "wq": wq_d[:].flatten_outer_dims(),
            "wv": wv_d[:].flatten_outer_dims(),
            "wp": wp_d[:],
        }
        # ---- weight load + XBAR transpose (load on SWDGE, XBAR on scalar) ----
        # wT4[p, r, b, s] = Wflat[r*128 + s, b*128 + p]
        def prep_w4(wname, pool):
            wT4 = pool.tile([P, 6, NCC, P], BF16, name=f"{wname}T4", tag="w4")
            for r in range(6):
                wbf = wbf_p.tile([P, C], BF16, tag="wbf", name=f"{wname}_bf{r}")
                nc.gpsimd.dma_start(out=wbf, in_=wflats[wname][r * P:(r + 1) * P, :])
                nc.scalar.dma_start_transpose(out=wT4[:, r, :, :], in_=wbf)
            return wT4

        # fp8 copies of wk/wq (x W8SC), on DVE, sliced per m for pipelining
        def cast_w8(wT4, name):
            w8 = sing.tile([P, 6, NCC, P], F8, name=name)
            for m in range(6):
                nc.vector.tensor_scalar_mul(w8[:, m], wT4[:, m], W8SC)
            return w8

        # ---- x loads (casting DMAs on the SWDGE queue) ----
        def p0a(ci):
            tok0 = ci * chunk_tok
            xbs = []
            for it in range(TT):
                xb = xbf_p.tile([P, C], BF16, tag="xbf")
                nc.gpsimd.dma_start(out=xb, in_=xf[tok0 + it * P:tok0 + (it + 1) * P, :])
                xbs.append(xb)
            return xbs

        # priority order: first chunk's QK needs wk/wq + x; wv/wp trail
        wkT4 = prep_w4("wk", w4_p)
        wqT4 = prep_w4("wq", w4_p)
        wk8 = cast_w8(wkT4, "wk8")
        wq8 = cast_w8(wqT4, "wq8")
        xbs_cur = p0a(0)

        mask_sb = sing.tile([P, 2, NHD, T], BF16)
        nc.sync.dma_start(out=mask_sb, in_=mk_d[:].rearrange(
            "p (two a b) -> p two a b", two=2, a=NHD))

        # ktbd zeros must land before chunk 0's K evacuation
        ktbd = sing.tile([P, NHD, NB, P], BF16, name="ktbd")
        nc.vector.memset(ktbd, 0.0)

        # wv/wp reuse the wk/wq bf16 buffers once the fp8 casts are done
        wvT4 = prep_w4("wv", w4_p)
        wpT4 = prep_w4("wp", w4_p)
        bp_bc = sing.tile([P, C], F32)
        nc.gpsimd.dma_start(out=bp_bc, in_=_bp_bcast_ap(bass, bp_d))
        # rhs views: [p, cc, m, s] ordering for V and O projections
        wvT4r = wvT4.rearrange("p m c s -> p c m s")
        wpT4r = wpT4.rearrange("p r j c -> p j r c")

        # ---- persistent block-diagonal V operand ----
        vbd = sing.tile([P, NHD, NB, 2 * (Dh + 1)], BF16, name="vbd")
        nc.vector.memset(vbd, 0.0)
        nc.vector.memset(vbd[0:T, :, :, Dh:Dh + 1], 1.0)
        nc.vector.memset(vbd[T:P, :, :, 2 * Dh + 1:2 * Dh + 2], 1.0)

        # ---- per-chunk X pipeline: XBAR transpose + fp8 cast ----
        # x8 is cc-major [c-part, cc, tok] so each QK psum bank accumulates
        # in ONE start/stop group with 512-wide streams
        def p0b(xbs):
            xT = xT_p.tile([P, TT, NCC, P], BF16, tag="xT")
            x8 = x8_p.tile([P, NCC, chunk_tok], F8, tag="x8")
            for it in range(TT):
                nc.scalar.dma_start_transpose(out=xT[:, it, :, :], in_=xbs[it])
            for it in range(TT):
                nc.vector.tensor_copy(
                    out=x8[:, :, it * P:(it + 1) * P], in_=xT[:, it])
            return xT, x8

        xT, x8 = p0b(xbs_cur)

        for ci in range(n_chunks):
            tok0 = ci * chunk_tok

            # next chunk X loads first: DMAs run during P1a, XBARs + fp8
            # casts (emitted below) land while sync/DVE queues are idle
            if ci + 1 < n_chunks:
                xbs_next = p0a(ci + 1)

            # ---- P1a: Q/K projections, fp8 DoubleRow ----
            qT = qT_p.tile([P, NHD, chunk_tok], BF16, tag="qT")
            nbsub = 512 // T
            for w8t, wT4t, dst in ((wk8, wkT4, "k"), (wq8, wqT4, "q")):
                for m in range(NHD):
                    for s in range(TT // 4):
                        psf = ptile(P, (P, 512), f"qk_{dst}{m}{s}")
                        for cc in range(F8CC):
                            nc.tensor.matmul(
                                psf, w8t[:, m, 2 * cc:2 * cc + 2, :],
                                x8[:, 2 * cc:2 * cc + 2, s * 512:(s + 1) * 512],
                                start=(cc == 0), stop=(cc == F8CC - 1),
                                perf_mode=DR)
                        if dst == "q":
                            nc.vector.tensor_copy(
                                out=qT[:, m, s * 512:(s + 1) * 512],
                                in_=psf)
                        else:
                            b0 = s * nbsub
                            nc.scalar.copy(
                                out=ktbd[0:T, m, b0:b0 + nbsub, 0:T],
                                in_=psf[0:T].rearrange(
                                    "p (nb t) -> p nb t", nb=nbsub))
                            nc.scalar.copy(
                                out=ktbd[T:P, m, b0:b0 + nbsub, T:P],
                                in_=psf[T:P].rearrange(
                                    "p (nb t) -> p nb t", nb=nbsub))

            if ci + 1 < n_chunks:
                xT_next, x8_next = p0b(xbs_next)

            # ---- P2a+P1b interleaved per tile: scores+exp+mask | V proj ----
            pex_all = pex_p.tile([P, NB, NHD, T], BF16, tag="pex")
            vbd_v = vbd.rearrange("p a (nb2 two) c -> p a nb2 two c", two=2)
            for it in range(TT):
                for half in range(2):
                    b = 2 * it + half
                    s_ps = ptile(P, (P, NHD, T), f"s_ps{b % 2}")
                    for m in range(NHD):
                        nc.tensor.matmul(
                            s_ps[:, m, :], ktbd[:, m, b, :],
                            qT[:, m, b * T:(b + 1) * T],
                            start=True, stop=True)
                    nc.scalar.activation(
                        out=pex_all[:, b], in_=s_ps, func=AF.Exp, scale=EXPSC)
                nc.vector.tensor_tensor(
                    pex_all[:, 2 * it:2 * it + 2], pex_all[:, 2 * it:2 * it + 2],
                    mask_sb, ALU.mult)
                # V projection for this tile
                psA = ptile(P, (P, 512), "v_psA")
                psB = ptile(P, (P, 256), "v_psB")
                for cc in range(NCC):
                    lhs = xT[:, it, cc, :]
                    nc.tensor.matmul(psA, lhs, wvT4r[:, cc, 0:4, :],
                                     start=(cc == 0), stop=(cc == NCC - 1))
                    nc.tensor.matmul(psB, lhs, wvT4r[:, cc, 4:6, :],
                                     start=(cc == 0), stop=(cc == NCC - 1))
                v_sb = vsb_p.tile([P, H, Dh], BF16, tag="v_sb")
                nc.scalar.copy(
                    out=v_sb[:, 0:8, :], in_=psA.rearrange("p (a b) -> p a b", a=8))
                nc.scalar.copy(
                    out=v_sb[:, 8:12, :], in_=psB.rearrange("p (a b) -> p a b", a=4))
                v_sb2 = v_sb.rearrange("p (h two) c -> p h two c", two=2)
                for par in range(2):
                    nc.sync.dma_start(
                        out=vbd_v[0:T, :, it, par, 0:Dh],
                        in_=v_sb2[par * T:(par + 1) * T, :, 0, :])
                    nc.sync.dma_start(
                        out=vbd_v[T:P, :, it, par, Dh + 1:2 * Dh + 1],
                        in_=v_sb2[par * T:(par + 1) * T, :, 1, :])

            # ---- P2b/P3: AV + normalize + XBAR-Y + output projection ----
            ybs = []
            ytiles = []
            def yt_tr(yb):
                # scalar HWDGE queue: compute-idle during P2b, and NOT behind
                # the vbd remap DMAs that congest the sync rings
                ytile = yt_p.tile([P, NHD, P], BF16, tag="ytile")
                nc.scalar.dma_start_transpose(out=ytile, in_=yb)
                ytiles.append(ytile)
            def oproj_emit(it):
                ytile = ytiles[it]
                oA = ptile(P, (P, 512), "o_psA")
                oB = ptile(P, (P, 256), "o_psB")
                for j in range(NHD):
                    lhs = ytile[:, j, :]
                    nc.tensor.matmul(oA, lhs, wpT4r[:, j, 0:4, :],
                                     start=(j == 0), stop=(j == NHD - 1))
                    nc.tensor.matmul(oB, lhs, wpT4r[:, j, 4:6, :],
                                     start=(j == 0), stop=(j == NHD - 1))
                osb = ostage.tile([P, C], F32, tag="osb")
                nc.vector.tensor_tensor(osb[:, 0:512], oA, bp_bc[:, 0:512], ALU.add)
                nc.vector.tensor_tensor(osb[:, 512:768], oB, bp_bc[:, 512:768], ALU.add)
                row0 = tok0 + it * P
                nc.gpsimd.dma_start(out=of[row0:row0 + P, :], in_=osb)

            for it in range(TT):
                yb = y_p.tile([P, HD], BF16, tag="yb")
                ybs.append(yb)
                y_ps = [ptile(P, (P, 3, 2 * (Dh + 1)), f"y_ps{h2}") for h2 in range(2)]
                for m in range(NHD):
                    for half in range(2):
                        b = it * 2 + half
                        prow = half * T
                        nc.tensor.matmul(
                            y_ps[m // 3][prow:prow + T, m % 3, :],
                            pex_all[:, b, m, :],
                            vbd[:, m, b, :],
                            start=True, stop=True)
                for h2 in range(2):
                    y_v = y_ps[h2].rearrange("p a (two c) -> p a two c", c=Dh + 1)
                    rec = small.tile([P, 3, 2, 1], F32, tag="rec", name="rec")
                    nc.vector.reciprocal(out=rec, in_=y_v[:, :, :, Dh:Dh + 1])
                    nc.vector.tensor_tensor(
                        yb[:, h2 * 384:(h2 + 1) * 384]
                            .rearrange("p (a two b) -> p a two b", a=3, two=2),
                        y_v[:, :, :, 0:Dh],
                        rec.to_broadcast([P, 3, 2, Dh]),
                        ALU.mult)
                if it >= 2:
                    yt_tr(ybs[it - 2])
                if it >= 3:
                    oproj_emit(it - 3)
            yt_tr(ybs[TT - 2])
            oproj_emit(TT - 3)
            yt_tr(ybs[TT - 1])
            oproj_emit(TT - 2)
            oproj_emit(TT - 1)
            if ci + 1 < n_chunks:
                xT, x8 = xT_next, x8_next

    _split_multi_waits(nc, mybir)
    return nc


def _get_program(B_loc, chunk_tok):
    key = (B_loc, chunk_tok)
    if key not in _cache:
        _cache[key] = build_nc(B_loc, chunk_tok)
    return _cache[key]


def make_const_inputs():
    import ml_dtypes
    # mask[s, t] = 1 if s <= t (causal, scoresT layout)
    m = np.tril(np.ones((T, T), dtype=np.float32)).T
    m2 = np.vstack([m, m])   # replicated for both partition-halves
    mask = np.tile(m2, (1, 2 * NHD)).astype(ml_dtypes.bfloat16)  # [P, 2*NHD*T]
    return mask


def prepare(x, Wq, Wk, Wv, Wp, bp, chunk_tok=1024):
    x = np.ascontiguousarray(x, dtype=np.float32)
    B_loc = B // N_CORES
    mask = make_const_inputs()
    nc = _get_program(B_loc, chunk_tok)
    in_maps = []
    for c in range(N_CORES):
        in_maps.append({
            "x": x[c * B_loc:(c + 1) * B_loc],
            "Wq": np.ascontiguousarray(Wq, dtype=np.float32),
            "Wk": np.ascontiguousarray(Wk, dtype=np.float32),
            "Wv": np.ascontiguousarray(Wv, dtype=np.float32),
            "Wp": np.ascontiguousarray(Wp, dtype=np.float32),
            "bp": np.ascontiguousarray(bp, dtype=np.float32),
            "mask": mask,
        })
    return nc, in_maps


def kernel(x, Wq, Wk, Wv, Wp, bp):
    from concourse import bass_utils

    nc, in_maps = prepare(x, Wq, Wk, Wv, Wp, bp)
    res = bass_utils.run_bass_kernel_spmd(nc, in_maps, list(range(N_CORES)))
    return np.concatenate([res.results[c]["out"] for c in range(N_CORES)], axis=0)


# revision 35
# speedup vs baseline: 1.6607x; 1.5866x over previous
"""Multi-head causal attention (B=1024, T=64, C=768, H=12, D=64) on 8 TRN2
NeuronCores, data-parallel over the batch dimension (128 batches/core).

Dataflow per core (all matmuls bf16, fp32 PSUM accumulate), processed in
1024-token chunks with coarse per-engine phases so the PE streams
back-to-back matmuls (no per-batch PE<->Scalar<->Vector ping-pong) and the
HAM clock-gate stays warm:
  - X tiles are loaded with SWDGE casting-DMAs (f32 DRAM -> bf16 SBUF, one
    chunk ahead) and PE-transposed (regular matmul vs a moving identity)
    into XT [c, tok].
  - P1a: KT then QT [hd, tok] = WT.T @ XT (weights stationary); KT is
    rewritten block-diagonally (2 heads per 128 partitions) into ktbd.
  - P2a immediately after: scoresT[s,t] = ktbd.T @ QT for all 16 batches;
    exp (Scalar, fused 1/8 scale) + causal-mask multiply (Vector) stage
    pex in SBUF while the PE moves on -- the exp/mask chain drains during
    the V projection, so AV never waits.
  - P1b: V in natural [tok, hd] layout, remapped per-tile by DMA into the
    block-diagonal Vaug (extra ones column -> softmax denominator falls out
    of the AV matmul for free).
  - P2b/P3 fused: AV matmuls Y[t, (h,d)|den] = pex.T @ Vaug with the two
    batch halves interleaved (disjoint PE column groups run concurrently);
    normalization (Vector), the Y transpose (PE, lag 2) and the output
    projection out[t, c] = YT.T @ WpT + bp (lag 3) are software-pipelined
    per tile -- the long projection streams keep PE duty high enough that
    the HAM clock-gate never re-throttles mid-chunk.
"""

import numpy as np

P = 128
B, T, C, H, Dh = 1024, 64, 768, 12, 64
HD = H * Dh            # 768
NCC = C // P           # 6 contraction chunks
NHD = HD // P          # 6 hd chunks
N_CORES = 8

_cache = {}


def _patch_tile_drain(tile, mybir):
    """walrus CTRL (Drain) ops in this toolchain accept only 1 sem-wait;
    spread the TileContext exit-drain's waits across preceding SP nops."""
    from concourse.vector_clock import ScopedClock

    if getattr(tile.TileContext, "_drain_patched", False):
        return

    def _drain_and_barrier(self, tick_clock, wait_clock):
        nc = self.nc
        drain_inst = nc.sync.drain()
        wait_clock.add_sem_waits(
            drain_inst.ins, ScopedClock({None: tick_clock.global_clock})
        )
        waits = list(drain_inst.ins.sync_info.on_wait)
        if len(waits) > 1:
            drain_inst.ins.sync_info.on_wait = waits[-1:]
            cur_bb = nc.cur_bb.bb
            idx = cur_bb.instructions.index(drain_inst.ins)
            extra = []
            for w in waits[:-1]:
                nop = mybir.InstNoOp(name=f"I-{nc.next_id()}", ins=[], outs=[])
                nop.engine = drain_inst.ins.engine
                nop.sync_info = mybir.SyncInfo(on_wait=[w], on_update=[])
                nc.register_instruction(nop)
                extra.append(nop)
            cur_bb.instructions[idx:idx] = extra
        nc.all_engine_barrier()
        assert self.sems is not None
        popped = nc._tile_sem_poison_stack.pop()
        assert popped is self._sem_poison
        nc.clear_and_free_semaphores(list(self.sems.allocated().values()))
        nc.all_engine_barrier()

    tile.TileContext._drain_and_barrier = _drain_and_barrier
    tile.TileContext._drain_patched = True


def _install_loud_cc_hook():
    """Surface real exceptions from the neuronx_cc hook (C wrapper eats them)."""
    from concourse import bass2jax as _b2j
    if getattr(_b2j, "_loud_hook_installed", False):
        return
    _orig = _b2j.neuronx_cc_hook
    def _loud(*a, **k):
        try:
            return _orig(*a, **k)
        except BaseException:
            import traceback
            traceback.print_exc()
            raise
    _b2j.neuronx_cc_hook = _loud
    _b2j._loud_hook_installed = True


def _split_multi_waits(nc, mybir, K=1):
    """This walrus build supports only one sem-wait per instruction: move
    excess waits onto same-engine NOPs inserted directly before the owner."""
    def fix_block(bb):
        insts = bb.instructions
        i = 0
        while i < len(insts):
            ins = insts[i]
            si = ins.sync_info
            w = list(si.on_wait) if si is not None and si.on_wait else []
            if len(w) > K:
                carriers = []
                for j in range(0, len(w) - K, K):
                    nop = mybir.InstNoOp(name=f"I-{nc.next_id()}", ins=[], outs=[])
                    nop.engine = ins.engine
                    nop.sync_info = mybir.SyncInfo(on_wait=w[j:j + K], on_update=[])
                    nc.register_instruction(nop)
                    carriers.append(nop)
                si.on_wait = w[len(w) - K:]
                insts[i:i] = carriers
                i += len(carriers)
            i += 1
    for fn in nc.m.functions:
        for bb in fn.blocks:
            fix_block(bb)


def _bp_bcast_ap(bass, bp_d):
    a = bp_d[:]
    return bass.AP(tensor=a.tensor, offset=a.offset, ap=[[0, P]] + list(a.ap))


def build_nc(B_loc=B // N_CORES, chunk_tok=1024, phases=6):
    import concourse.bass as bass
    import concourse.tile as tile
    from concourse import mybir
    from contextlib import ExitStack

    _patch_tile_drain(tile, mybir)
    _install_loud_cc_hook()

    F32 = mybir.dt.float32
    BF16 = mybir.dt.bfloat16
    F8 = mybir.dt.float8e4
    AF = mybir.ActivationFunctionType
    ALU = mybir.AluOpType
    DR = mybir.MatmulPerfMode.DoubleRow

    BT = B_loc * T
    chunk_tok = min(chunk_tok, BT)
    n_chunks = BT // chunk_tok
    assert n_chunks * chunk_tok == BT
    TT = chunk_tok // P     # 128-token tiles per chunk
    NB = chunk_tok // T     # batches per chunk

    # Q/K projections run in fp8(e4m3) DoubleRow perf mode: 2 contraction
    # rows per partition per cycle (halves the QK streaming time). W is
    # pre-scaled by 2^10 into fp8 (keeps 0.02-std weights out of the
    # subnormal range); the 2^-20 descale folds into the exp scale.
    W8SC = 1024.0
    EXPSC = 0.125 * (2.0 ** -20)

    nc = bass.Bass()
    x_d = nc.declare_dram_parameter("x", [B_loc, T, C], F32, isOutput=False)
    wq_d = nc.declare_dram_parameter("Wq", [H, Dh, C], F32, isOutput=False)
    wk_d = nc.declare_dram_parameter("Wk", [H, Dh, C], F32, isOutput=False)
    wv_d = nc.declare_dram_parameter("Wv", [H, Dh, C], F32, isOutput=False)
    wp_d = nc.declare_dram_parameter("Wp", [C, HD], F32, isOutput=False)
    bp_d = nc.declare_dram_parameter("bp", [C], F32, isOutput=False)
    id_d = nc.declare_dram_parameter("ident", [P, P], BF16, isOutput=False)
    mk_d = nc.declare_dram_parameter("mask", [P, 2 * NHD * T], BF16, isOutput=False)
    out_d = nc.declare_dram_parameter("out", [B_loc, T, C], F32, isOutput=True)

    xf = x_d[:].flatten_outer_dims()      # [BT, C]
    of = out_d[:].flatten_outer_dims()    # [BT, C]

    with tile.TileContext(nc) as tc, ExitStack() as ctx:
        sing = ctx.enter_context(tc.tile_pool(name="sing", bufs=1))
        xbf_p = ctx.enter_context(tc.tile_pool(name="xbfp", bufs=8))
        wbf_p = ctx.enter_context(tc.tile_pool(name="wbfp", bufs=3))
        ostage = ctx.enter_context(tc.tile_pool(name="ostage", bufs=2))
        xT_p = ctx.enter_context(tc.tile_pool(name="xTp", bufs=2))
        x8_p = ctx.enter_context(tc.tile_pool(name="x8p", bufs=2))
        qT_p = ctx.enter_context(tc.tile_pool(name="qTp", bufs=1))
        v_p = ctx.enter_context(tc.tile_pool(name="vp", bufs=1))
        pex_p = ctx.enter_context(tc.tile_pool(name="pex", bufs=1))
        y_p = ctx.enter_context(tc.tile_pool(name="y", bufs=4))
        yt_p = ctx.enter_context(tc.tile_pool(name="yt", bufs=6))
        small = ctx.enter_context(tc.tile_pool(name="small", bufs=6))
        pp = ctx.enter_context(tc.tile_pool(name="pp", bufs=8, space="PSUM"))

        def ptile(pdim, shape, name, dt=None):
            # all PSUM tiles share one 1-bank slot class: within a phase this
            # gives maximum in-flight tiles (a per-phase split starves P1a)
            t = pp.tile([P, 512], dt or F32, tag="ps", name=name)
            flat = t[:pdim, : int(np.prod(shape[1:]))]
            return flat.rearrange(
                "p (a b) -> p a b", a=shape[1]
            ) if len(shape) == 3 else flat

        def tr128(dst, src):
            # dst = src.T as a REGULAR matmul (normal LDW + moving identity):
            # cheap on the PE and counts as PE activity for the HAM clock-gate
            nc.tensor.matmul(dst, src, id_sb, start=True, stop=True)

        # ---- constants ----
        id_sb = sing.tile([P, P], BF16)
        nc.sync.dma_start(out=id_sb, in_=id_d[:])
        mask_sb = sing.tile([P, 2, NHD, T], BF16)
        nc.sync.dma_start(out=mask_sb, in_=mk_d[:].rearrange(
            "p (two a b) -> p two a b", two=2, a=NHD))
        bp_bc = sing.tile([P, C], F32)
        nc.gpsimd.dma_start(
            out=bp_bc,
            in_=_bp_bcast_ap(bass, bp_d),
        )

        # ---- persistent block-diagonal attention operands ----
        # (memsets moved off the gpsimd queue -- they were serializing ahead
        # of the weight-DMA dispatches there -- and emitted after the wk/wq
        # loads below so the DMA queue starts immediately)
        NBmax = chunk_tok // T
        ktbd = sing.tile([P, NHD, NBmax, P], BF16, name="ktbd")
        vbd = sing.tile([P, NHD, NBmax, 2 * (Dh + 1)], BF16, name="vbd")

        # ---- weight prep: WT_sb[col_in, col_out, row] = flat[row, col]^T ----
        def prep_wT(wflat, name):
            wT = sing.tile([P, NCC, 768], BF16, name=name)
            for r in range(6):
                wbf = wbf_p.tile([P, 768], BF16, tag="wbf")
                # SWDGE casting DMA: f32 DRAM -> bf16 SBUF in one shot
                nc.gpsimd.dma_start(out=wbf, in_=wflat[r * P:(r + 1) * P, :])
                psA = ptile(P, (P, 4, P), f"{name}_psA_{r}")
                psB = ptile(P, (P, 2, P), f"{name}_psB_{r}")
                for j in range(6):
                    dst = psA[:, j, :] if j < 4 else psB[:, j - 4, :]
                    tr128(dst, wbf[:, j * P:(j + 1) * P])
                nc.vector.tensor_copy(
                    out=wT[:, 0:4, r * P:(r + 1) * P], in_=psA)
                nc.vector.tensor_copy(
                    out=wT[:, 4:6, r * P:(r + 1) * P], in_=psB)
            return wT

        def p0a(ci):
            # casting loads (SWDGE) for chunk ci, emitted one chunk early
            tok0 = ci * chunk_tok
            xbs = []
            for it in range(TT):
                row0 = tok0 + it * P
                xb = xbf_p.tile([P, C], BF16, tag="xbf")
                nc.gpsimd.dma_start(out=xb, in_=xf[row0:row0 + P, :])
                xbs.append(xb)
            return xbs

        def p0b(xbs):
            # PE transposes + vector evac into xT; runs mid-chunk so xT is
            # complete well before the next chunk's projections need it.
            # x8 is the fp8 copy feeding the DoubleRow Q/K projections.
            xT = xT_p.tile([P, NCC, chunk_tok], BF16, tag="xT")
            x8 = x8_p.tile([P, NCC, chunk_tok], F8, tag="x8")
            for it in range(TT):
                psA = ptile(P, (P, 4, P), "x_psA")
                psB = ptile(P, (P, 2, P), "x_psB")
                for j in range(6):
                    dst = psA[:, j, :] if j < 4 else psB[:, j - 4, :]
                    tr128(dst, xbs[it][:, j * P:(j + 1) * P])
                nc.vector.tensor_copy(out=xT[:, 0:4, it * P:(it + 1) * P], in_=psA)
                nc.vector.tensor_copy(out=xT[:, 4:6, it * P:(it + 1) * P], in_=psB)
                nc.vector.tensor_copy(
                    out=x8[:, :, it * P:(it + 1) * P],
                    in_=xT[:, :, it * P:(it + 1) * P])
            return xT, x8

        def cast_w8(wT, name):
            w8 = sing.tile([P, NCC, 768], F8, name=name)
            for cc in range(NCC):
                nc.vector.tensor_scalar_mul(w8[:, cc], wT[:, cc], W8SC)
            return w8

        # prologue order follows the first chunk's dependency chain: the
        # K projection needs wkT + xT(0) first, Q next; wv/wp can trail
        wkT = prep_wT(wk_d[:].flatten_outer_dims(), "wkT")
        wqT = prep_wT(wq_d[:].flatten_outer_dims(), "wqT")
        wk8 = cast_w8(wkT, "wk8")
        wq8 = cast_w8(wqT, "wq8")
        nc.vector.memset(ktbd, 0.0)
        nc.vector.memset(vbd, 0.0)
        nc.vector.memset(vbd[0:T, :, :, Dh:Dh + 1], 1.0)
        nc.vector.memset(vbd[T:P, :, :, 2 * Dh + 1:2 * Dh + 2], 1.0)
        wvT = prep_wT(wv_d[:].flatten_outer_dims(), "wvT")
        wpT = prep_wT(wp_d[:], "wpT")
        xbs0 = p0a(0)

        xT, x8 = p0b(xbs0)
        for ci in range(n_chunks):
            tok0 = ci * chunk_tok

            # ---- P1a: QT / KT projections (weights stationary) ----
            SUBW = min(512, chunk_tok)
            n_sub = chunk_tok // SUBW
            qT = qT_p.tile([P, NHD, chunk_tok], BF16, tag="qT")
            nbsub = SUBW // T          # batches per SUBW token block
            for w8t, dst in ((wk8, "k"), (wq8, "q")):
                for m in range(NHD):
                    pss = [ptile(P, (P, SUBW), f"proj_{m}_{s}") for s in range(n_sub)]
                    for cc in range(NCC // 2):
                        lhs = w8t[:, 2 * cc:2 * cc + 2, m * P:(m + 1) * P]
                        for s in range(n_sub):
                            nc.tensor.matmul(
                                pss[s], lhs,
                                x8[:, 2 * cc:2 * cc + 2, s * SUBW:(s + 1) * SUBW],
                                start=(cc == 0), stop=(cc == NCC // 2 - 1),
                                perf_mode=DR)
                    for s in range(n_sub):
                        if dst == "q":
                            nc.vector.tensor_copy(
                                out=qT[:, m, s * SUBW:(s + 1) * SUBW], in_=pss[s])
                        else:
                            b0 = s * nbsub
                            nc.vector.tensor_copy(
                                out=ktbd[0:T, m, b0:b0 + nbsub, 0:T],
                                in_=pss[s][0:T].rearrange(
                                    "p (nb t) -> p nb t", nb=nbsub))
                            nc.vector.tensor_copy(
                                out=ktbd[T:P, m, b0:b0 + nbsub, T:P],
                                in_=pss[s][T:P].rearrange(
                                    "p (nb t) -> p nb t", nb=nbsub))
            # ---- P2a: scores + exp + mask (right after QK so the exp/mask
            # chain completes during the V projection) ----
            pex_all = pex_p.tile([P, NB, NHD, T], BF16, tag="pex")
            for b in range(NB):
                bt0 = b * T
                s_ps = ptile(P, (P, NHD, T), f"s_ps{b % 2}")
                for p_ in range(NHD):
                    nc.tensor.matmul(
                        s_ps[:, p_, :],
                        ktbd[:, p_, b, :],
                        qT[:, p_, bt0:bt0 + T],
                        start=True, stop=True)
                nc.scalar.activation(
                    out=pex_all[:, b], in_=s_ps, func=AF.Exp, scale=EXPSC)
                if b % 2 == 1:
                    nc.vector.tensor_tensor(
                        pex_all[:, b - 1:b + 1], pex_all[:, b - 1:b + 1],
                        mask_sb, ALU.mult)
            # ---- P1b: V in natural layout, block-diag via PER-TILE remaps ----
            v_sb = v_p.tile([P, TT, H, Dh], BF16, tag="v_sb")
            v_sb2 = v_sb.rearrange("p a (h two) c -> p a h two c", two=2)
            vbd_v = vbd.rearrange("p a (nb2 two) c -> p a nb2 two c", two=2)
            for it in range(TT):
                psA = ptile(P, (P, 512), "v_psA")
                psB = ptile(P, (P, 256), "v_psB")
                for cc in range(NCC):
                    lhs = xT[:, cc, it * P:(it + 1) * P]
                    nc.tensor.matmul(psA, lhs, wvT[:, cc, 0:512],
                                     start=(cc == 0), stop=(cc == NCC - 1))
                    nc.tensor.matmul(psB, lhs, wvT[:, cc, 512:768],
                                     start=(cc == 0), stop=(cc == NCC - 1))
                nc.scalar.copy(
                    out=v_sb[:, it, 0:8, :],
                    in_=psA.rearrange("p (a b) -> p a b", a=8))
                nc.scalar.copy(
                    out=v_sb[:, it, 8:12, :],
                    in_=psB.rearrange("p (a b) -> p a b", a=4))
                for par in range(2):
                    nc.sync.dma_start(
                        out=vbd_v[0:T, :, it, par, 0:Dh],
                        in_=v_sb2[par * T:(par + 1) * T, it, :, 0, :])
                    nc.sync.dma_start(
                        out=vbd_v[T:P, :, it, par, Dh + 1:2 * Dh + 1],
                        in_=v_sb2[par * T:(par + 1) * T, it, :, 1, :])

            # ---- next chunk X loads (early; cheap SWDGE casting DMAs) ----
            if ci + 1 < n_chunks:
                xbs_next = p0a(ci + 1)

            # ---- P2b: AV + normalization + Y transpose, pipelined per tile
            ytiles = []
            def yt_tr(yb):
                ytA = ptile(P, (P, 4, P), "yt_psA")
                ytB = ptile(P, (P, 2, P), "yt_psB")
                for j in range(6):
                    dst = ytA[:, j, :] if j < 4 else ytB[:, j - 4, :]
                    tr128(dst, yb[:, j * P:(j + 1) * P])
                ytile = yt_p.tile([P, NHD, P], BF16, tag="ytile")
                nc.scalar.copy(out=ytile[:, 0:4, :], in_=ytA)
                nc.scalar.copy(out=ytile[:, 4:6, :], in_=ytB)
                ytiles.append(ytile)
            def oproj_emit(it):
                ytile = ytiles[it]
                oA = ptile(P, (P, 512), "o_psA")
                oB = ptile(P, (P, 256), "o_psB")
                for j in range(NHD):
                    lhs = ytile[:, j, :]
                    nc.tensor.matmul(oA, lhs, wpT[:, j, 0:512],
                                     start=(j == 0), stop=(j == NHD - 1))
                    nc.tensor.matmul(oB, lhs, wpT[:, j, 512:768],
                                     start=(j == 0), stop=(j == NHD - 1))
                osb = ostage.tile([P, C], F32, tag="osb")
                nc.vector.tensor_tensor(osb[:, 0:512], oA, bp_bc[:, 0:512], ALU.add)
                nc.vector.tensor_tensor(osb[:, 512:768], oB, bp_bc[:, 512:768], ALU.add)
                row0 = tok0 + it * P
                nc.sync.dma_start(out=of[row0:row0 + P, :], in_=osb)

            # AV, normalize, Y-transpose and output projection software-
            # pipelined per tile: the long oproj matmul streams keep the PE
            # duty high so the HAM clock-gate never re-throttles mid-chunk
            ybs = []
            for it in range(TT):
                yb = y_p.tile([P, HD], BF16, tag="yb")
                ybs.append(yb)
                y_ps = [ptile(P, (P, 3, 2 * (Dh + 1)), f"y_ps{h2}") for h2 in range(2)]
                for p_ in range(NHD):
                    for half in range(2):      # two batches per 128-token tile
                        b = it * 2 + half
                        prow = half * T
                        nc.tensor.matmul(
                            y_ps[p_ // 3][prow:prow + T, p_ % 3, :],
                            pex_all[:, b, p_, :],
                            vbd[:, p_, b, :],
                            start=True, stop=True)
                for h2 in range(2):
                    y_v = y_ps[h2].rearrange("p a (two c) -> p a two c", c=Dh + 1)
                    rec = small.tile([P, 3, 2, 1], F32, tag="rec", name="rec")
                    nc.vector.reciprocal(out=rec, in_=y_v[:, :, :, Dh:Dh + 1])
                    nc.vector.tensor_tensor(
                        yb[:, h2 * 384:(h2 + 1) * 384]
                            .rearrange("p (a two b) -> p a two b", a=3, two=2),
                        y_v[:, :, :, 0:Dh],
                        rec.to_broadcast([P, 3, 2, Dh]),
                        ALU.mult)
                if it >= 2:
                    yt_tr(ybs[it - 2])
                if it >= 3:
                    oproj_emit(it - 3)
            yt_tr(ybs[TT - 2])
            oproj_emit(TT - 3)
            yt_tr(ybs[TT - 1])
            oproj_emit(TT - 2)
            if ci + 1 < n_chunks:
                xT_next, x8_next = p0b(xbs_next)
            oproj_emit(TT - 1)
            if ci + 1 < n_chunks:
                xT, x8 = xT_next, x8_next

    _split_multi_waits(nc, mybir)
    return nc


def _get_program(B_loc, chunk_tok):
    key = (B_loc, chunk_tok)
    if key not in _cache:
        _cache[key] = build_nc(B_loc, chunk_tok)
    return _cache[key]


def make_const_inputs():
    import ml_dtypes
    ident = np.eye(P, dtype=ml_dtypes.bfloat16)
    # mask[s, t] = 1 if s <= t (causal, scoresT layout)
    m = np.tril(np.ones((T, T), dtype=np.float32)).T
    m2 = np.vstack([m, m])   # replicated for both partition-halves
    mask = np.tile(m2, (1, 2 * NHD)).astype(ml_dtypes.bfloat16)  # [P, 2*NHD*T]
    return ident, mask


def prepare(x, Wq, Wk, Wv, Wp, bp, chunk_tok=1024):
    x = np.ascontiguousarray(x, dtype=np.float32)
    B_loc = B // N_CORES
    ident, mask = make_const_inputs()
    nc = _get_program(B_loc, chunk_tok)
    in_maps = []
    for c in range(N_CORES):
        in_maps.append({
            "x": x[c * B_loc:(c + 1) * B_loc],
            "Wq": np.ascontiguousarray(Wq, dtype=np.float32),
            "Wk": np.ascontiguousarray(Wk, dtype=np.float32),
            "Wv": np.ascontiguousarray(Wv, dtype=np.float32),
            "Wp": np.ascontiguousarray(Wp, dtype=np.float32),
            "bp": np.ascontiguousarray(bp, dtype=np.float32),
            "ident": ident,
            "mask": mask,
        })
    return nc, in_maps


def kernel(x, Wq, Wk, Wv, Wp, bp):
    from concourse import bass_utils

    nc, in_maps = prepare(x, Wq, Wk, Wv, Wp, bp)
    res = bass_utils.run_bass_kernel_spmd(nc, in_maps, list(range(N_CORES)))
    return np.concatenate([res.results[c]["out"] for c in range(N_CORES)], axis=0)



# revision 38
# speedup vs baseline: 1.8332x; 1.1039x over previous
"""Multi-head causal attention (B=1024, T=64, C=768, H=12, D=64) on 8 TRN2
NeuronCores, data-parallel over the batch dimension (128 batches/core).

Dataflow per core (all matmuls bf16, fp32 PSUM accumulate), processed in
1024-token chunks with coarse per-engine phases so the PE streams
back-to-back matmuls (no per-batch PE<->Scalar<->Vector ping-pong) and the
HAM clock-gate stays warm:
  - X tiles are loaded with SWDGE casting-DMAs (f32 DRAM -> bf16 SBUF, one
    chunk ahead) and PE-transposed (regular matmul vs a moving identity)
    into XT [c, tok].
  - P1a: KT then QT [hd, tok] = WT.T @ XT (weights stationary); KT is
    rewritten block-diagonally (2 heads per 128 partitions) into ktbd.
  - P2a immediately after: scoresT[s,t] = ktbd.T @ QT for all 16 batches;
    exp (Scalar, fused 1/8 scale) + causal-mask multiply (Vector) stage
    pex in SBUF while the PE moves on -- the exp/mask chain drains during
    the V projection, so AV never waits.
  - P1b: V in natural [tok, hd] layout, remapped per-tile by DMA into the
    block-diagonal Vaug (extra ones column -> softmax denominator falls out
    of the AV matmul for free).
  - P2b/P3 fused: AV matmuls Y[t, (h,d)|den] = pex.T @ Vaug with the two
    batch halves interleaved (disjoint PE column groups run concurrently);
    normalization (Vector), the Y transpose (PE, lag 2) and the output
    projection out[t, c] = YT.T @ WpT + bp (lag 3) are software-pipelined
    per tile -- the long projection streams keep PE duty high enough that
    the HAM clock-gate never re-throttles mid-chunk.
"""

import numpy as np

P = 128
B, T, C, H, Dh = 1024, 64, 768, 12, 64
HD = H * Dh            # 768
NCC = C // P           # 6 contraction chunks
NHD = HD // P          # 6 hd chunks
N_CORES = 8

_cache = {}


def _patch_tile_drain(tile, mybir):
    """walrus CTRL (Drain) ops in this toolchain accept only 1 sem-wait;
    spread the TileContext exit-drain's waits across preceding SP nops."""
    from concourse.vector_clock import ScopedClock

    if getattr(tile.TileContext, "_drain_patched", False):
        return

    def _drain_and_barrier(self, tick_clock, wait_clock):
        nc = self.nc
        drain_inst = nc.sync.drain()
        wait_clock.add_sem_waits(
            drain_inst.ins, ScopedClock({None: tick_clock.global_clock})
        )
        waits = list(drain_inst.ins.sync_info.on_wait)
        if len(waits) > 1:
            drain_inst.ins.sync_info.on_wait = waits[-1:]
            cur_bb = nc.cur_bb.bb
            idx = cur_bb.instructions.index(drain_inst.ins)
            extra = []
            for w in waits[:-1]:
                nop = mybir.InstNoOp(name=f"I-{nc.next_id()}", ins=[], outs=[])
                nop.engine = drain_inst.ins.engine
                nop.sync_info = mybir.SyncInfo(on_wait=[w], on_update=[])
                nc.register_instruction(nop)
                extra.append(nop)
            cur_bb.instructions[idx:idx] = extra
        nc.all_engine_barrier()
        assert self.sems is not None
        popped = nc._tile_sem_poison_stack.pop()
        assert popped is self._sem_poison
        nc.clear_and_free_semaphores(list(self.sems.allocated().values()))
        nc.all_engine_barrier()

    tile.TileContext._drain_and_barrier = _drain_and_barrier
    tile.TileContext._drain_patched = True


def _install_loud_cc_hook():
    """Surface real exceptions from the neuronx_cc hook (C wrapper eats them)."""
    from concourse import bass2jax as _b2j
    if getattr(_b2j, "_loud_hook_installed", False):
        return
    _orig = _b2j.neuronx_cc_hook
    def _loud(*a, **k):
        try:
            return _orig(*a, **k)
        except BaseException:
            import traceback
            traceback.print_exc()
            raise
    _b2j.neuronx_cc_hook = _loud
    _b2j._loud_hook_installed = True


def _split_multi_waits(nc, mybir, K=1):
    """This walrus build supports only one sem-wait per instruction: move
    excess waits onto same-engine NOPs inserted directly before the owner."""
    def fix_block(bb):
        insts = bb.instructions
        i = 0
        while i < len(insts):
            ins = insts[i]
            si = ins.sync_info
            w = list(si.on_wait) if si is not None and si.on_wait else []
            if len(w) > K:
                carriers = []
                for j in range(0, len(w) - K, K):
                    nop = mybir.InstNoOp(name=f"I-{nc.next_id()}", ins=[], outs=[])
                    nop.engine = ins.engine
                    nop.sync_info = mybir.SyncInfo(on_wait=w[j:j + K], on_update=[])
                    nc.register_instruction(nop)
                    carriers.append(nop)
                si.on_wait = w[len(w) - K:]
                insts[i:i] = carriers
                i += len(carriers)
            i += 1
    for fn in nc.m.functions:
        for bb in fn.blocks:
            fix_block(bb)


def _bp_bcast_ap(bass, bp_d):
    a = bp_d[:]
    return bass.AP(tensor=a.tensor, offset=a.offset, ap=[[0, P]] + list(a.ap))


def build_nc(B_loc=B // N_CORES, chunk_tok=1024, phases=6):
    import concourse.bass as bass
    import concourse.tile as tile
    from concourse import mybir
    from contextlib import ExitStack

    _patch_tile_drain(tile, mybir)
    _install_loud_cc_hook()

    F32 = mybir.dt.float32
    BF16 = mybir.dt.bfloat16
    F8 = mybir.dt.float8e4
    AF = mybir.ActivationFunctionType
    ALU = mybir.AluOpType
    DR = mybir.MatmulPerfMode.DoubleRow

    BT = B_loc * T
    chunk_tok = min(chunk_tok, BT)
    n_chunks = BT // chunk_tok
    assert n_chunks * chunk_tok == BT
    TT = chunk_tok // P     # 128-token tiles per chunk
    NB = chunk_tok // T     # batches per chunk

    # Q/K projections run in fp8(e4m3) DoubleRow perf mode: 2 contraction
    # rows per partition per cycle (halves the QK streaming time). W is
    # pre-scaled by 2^10 into fp8 (keeps 0.02-std weights out of the
    # subnormal range); the 2^-20 descale folds into the exp scale.
    W8SC = 1024.0
    EXPSC = 0.125 * (2.0 ** -20)

    nc = bass.Bass()
    x_d = nc.declare_dram_parameter("x", [B_loc, T, C], F32, isOutput=False)
    wq_d = nc.declare_dram_parameter("Wq", [H, Dh, C], F32, isOutput=False)
    wk_d = nc.declare_dram_parameter("Wk", [H, Dh, C], F32, isOutput=False)
    wv_d = nc.declare_dram_parameter("Wv", [H, Dh, C], F32, isOutput=False)
    wp_d = nc.declare_dram_parameter("Wp", [C, HD], F32, isOutput=False)
    bp_d = nc.declare_dram_parameter("bp", [C], F32, isOutput=False)
    id_d = nc.declare_dram_parameter("ident", [P, P], BF16, isOutput=False)
    mk_d = nc.declare_dram_parameter("mask", [P, 2 * NHD * T], BF16, isOutput=False)
    out_d = nc.declare_dram_parameter("out", [B_loc, T, C], F32, isOutput=True)

    xf = x_d[:].flatten_outer_dims()      # [BT, C]
    of = out_d[:].flatten_outer_dims()    # [BT, C]

    with tile.TileContext(nc) as tc, ExitStack() as ctx:
        sing = ctx.enter_context(tc.tile_pool(name="sing", bufs=1))
        xbf_p = ctx.enter_context(tc.tile_pool(name="xbfp", bufs=8))
        wbf_p = ctx.enter_context(tc.tile_pool(name="wbfp", bufs=3))
        ostage = ctx.enter_context(tc.tile_pool(name="ostage", bufs=2))
        xT_p = ctx.enter_context(tc.tile_pool(name="xTp", bufs=2))
        x8_p = ctx.enter_context(tc.tile_pool(name="x8p", bufs=2))
        qT_p = ctx.enter_context(tc.tile_pool(name="qTp", bufs=1))
        v_p = ctx.enter_context(tc.tile_pool(name="vp", bufs=1))
        pex_p = ctx.enter_context(tc.tile_pool(name="pex", bufs=1))
        y_p = ctx.enter_context(tc.tile_pool(name="y", bufs=4))
        yt_p = ctx.enter_context(tc.tile_pool(name="yt", bufs=6))
        small = ctx.enter_context(tc.tile_pool(name="small", bufs=6))
        pp = ctx.enter_context(tc.tile_pool(name="pp", bufs=8, space="PSUM"))

        def ptile(pdim, shape, name, dt=None):
            # all PSUM tiles share one 1-bank slot class: within a phase this
            # gives maximum in-flight tiles (a per-phase split starves P1a)
            t = pp.tile([P, 512], dt or F32, tag="ps", name=name)
            flat = t[:pdim, : int(np.prod(shape[1:]))]
            return flat.rearrange(
                "p (a b) -> p a b", a=shape[1]
            ) if len(shape) == 3 else flat

        def tr128(dst, src):
            # dst = src.T as a REGULAR matmul (normal LDW + moving identity):
            # cheap on the PE and counts as PE activity for the HAM clock-gate
            nc.tensor.matmul(dst, src, id_sb, start=True, stop=True)

        # ---- constants ----
        id_sb = sing.tile([P, P], BF16)
        nc.sync.dma_start(out=id_sb, in_=id_d[:])
        mask_sb = sing.tile([P, 2, NHD, T], BF16)
        nc.sync.dma_start(out=mask_sb, in_=mk_d[:].rearrange(
            "p (two a b) -> p two a b", two=2, a=NHD))
        bp_bc = sing.tile([P, C], F32)
        nc.gpsimd.dma_start(
            out=bp_bc,
            in_=_bp_bcast_ap(bass, bp_d),
        )

        # ---- persistent block-diagonal attention operands ----
        # (memsets moved off the gpsimd queue -- they were serializing ahead
        # of the weight-DMA dispatches there -- and emitted after the wk/wq
        # loads below so the DMA queue starts immediately)
        NBmax = chunk_tok // T
        ktbd = sing.tile([P, NHD, NBmax, P], BF16, name="ktbd")
        vbd = sing.tile([P, NHD, NBmax, 2 * (Dh + 1)], BF16, name="vbd")

        # ---- weight prep: WT_sb[col_in, col_out, row] = flat[row, col]^T ----
        def prep_wT(wflat, name):
            wT = sing.tile([P, NCC, 768], BF16, name=name)
            for r in range(6):
                wbf = wbf_p.tile([P, 768], BF16, tag="wbf")
                # SWDGE casting DMA: f32 DRAM -> bf16 SBUF in one shot
                nc.gpsimd.dma_start(out=wbf, in_=wflat[r * P:(r + 1) * P, :])
                psA = ptile(P, (P, 4, P), f"{name}_psA_{r}")
                psB = ptile(P, (P, 2, P), f"{name}_psB_{r}")
                for j in range(6):
                    dst = psA[:, j, :] if j < 4 else psB[:, j - 4, :]
                    tr128(dst, wbf[:, j * P:(j + 1) * P])
                nc.vector.tensor_copy(
                    out=wT[:, 0:4, r * P:(r + 1) * P], in_=psA)
                nc.vector.tensor_copy(
                    out=wT[:, 4:6, r * P:(r + 1) * P], in_=psB)
            return wT

        def p0a(ci):
            # casting loads (SWDGE) for chunk ci, emitted one chunk early
            tok0 = ci * chunk_tok
            xbs = []
            for it in range(TT):
                row0 = tok0 + it * P
                xb = xbf_p.tile([P, C], BF16, tag="xbf")
                nc.gpsimd.dma_start(out=xb, in_=xf[row0:row0 + P, :])
                xbs.append(xb)
            return xbs

        def p0b(xbs):
            # PE transposes + vector evac into xT; runs mid-chunk so xT is
            # complete well before the next chunk's projections need it.
            # x8 is the fp8 copy feeding the DoubleRow Q/K projections.
            xT = xT_p.tile([P, NCC, chunk_tok], BF16, tag="xT")
            x8 = x8_p.tile([P, NCC, chunk_tok], F8, tag="x8")
            for it in range(TT):
                psA = ptile(P, (P, 4, P), "x_psA")
                psB = ptile(P, (P, 2, P), "x_psB")
                for j in range(6):
                    dst = psA[:, j, :] if j < 4 else psB[:, j - 4, :]
                    tr128(dst, xbs[it][:, j * P:(j + 1) * P])
                nc.vector.tensor_copy(out=xT[:, 0:4, it * P:(it + 1) * P], in_=psA)
                nc.vector.tensor_copy(out=xT[:, 4:6, it * P:(it + 1) * P], in_=psB)
                nc.vector.tensor_copy(
                    out=x8[:, :, it * P:(it + 1) * P],
                    in_=xT[:, :, it * P:(it + 1) * P])
            return xT, x8

        def cast_w8(wT, name):
            w8 = sing.tile([P, NCC, 768], F8, name=name)
            for cc in range(NCC):
                nc.vector.tensor_scalar_mul(w8[:, cc], wT[:, cc], W8SC)
            return w8

        # prologue order follows the first chunk's dependency chain: the
        # K projection needs wkT + xT(0) first, Q next; wv/wp can trail.
        # ktbd zeros ride DVE while the first weight DMA is in flight; vbd
        # zeros ride gpsimd after all prologue DMA dispatches (first vbd
        # write is the chunk-0 V remap, ~35us in).
        nc.vector.memset(ktbd, 0.0)
        wkT = prep_wT(wk_d[:].flatten_outer_dims(), "wkT")
        wqT = prep_wT(wq_d[:].flatten_outer_dims(), "wqT")
        wk8 = cast_w8(wkT, "wk8")
        wq8 = cast_w8(wqT, "wq8")
        wvT = prep_wT(wv_d[:].flatten_outer_dims(), "wvT")
        wpT = prep_wT(wp_d[:], "wpT")
        xbs0 = p0a(0)
        nc.gpsimd.memset(vbd, 0.0)
        nc.gpsimd.memset(vbd[0:T, :, :, Dh:Dh + 1], 1.0)
        nc.gpsimd.memset(vbd[T:P, :, :, 2 * Dh + 1:2 * Dh + 2], 1.0)

        xT, x8 = p0b(xbs0)
        for ci in range(n_chunks):
            tok0 = ci * chunk_tok

            # ---- P1a: QT / KT projections (weights stationary) ----
            SUBW = min(512, chunk_tok)
            n_sub = chunk_tok // SUBW
            qT = qT_p.tile([P, NHD, chunk_tok], BF16, tag="qT")
            nbsub = SUBW // T          # batches per SUBW token block
            for w8t, dst in ((wk8, "k"), (wq8, "q")):
                for m in range(NHD):
                    pss = [ptile(P, (P, SUBW), f"proj_{m}_{s}") for s in range(n_sub)]
                    for cc in range(NCC // 2):
                        lhs = w8t[:, 2 * cc:2 * cc + 2, m * P:(m + 1) * P]
                        for s in range(n_sub):
                            nc.tensor.matmul(
                                pss[s], lhs,
                                x8[:, 2 * cc:2 * cc + 2, s * SUBW:(s + 1) * SUBW],
                                start=(cc == 0), stop=(cc == NCC // 2 - 1),
                                perf_mode=DR)
                    for s in range(n_sub):
                        if dst == "q":
                            nc.vector.tensor_copy(
                                out=qT[:, m, s * SUBW:(s + 1) * SUBW], in_=pss[s])
                        else:
                            b0 = s * nbsub
                            nc.scalar.copy(
                                out=ktbd[0:T, m, b0:b0 + nbsub, 0:T],
                                in_=pss[s][0:T].rearrange(
                                    "p (nb t) -> p nb t", nb=nbsub))
                            nc.scalar.copy(
                                out=ktbd[T:P, m, b0:b0 + nbsub, T:P],
                                in_=pss[s][T:P].rearrange(
                                    "p (nb t) -> p nb t", nb=nbsub))
            # ---- P2a: scores + exp + mask (right after QK so the exp/mask
            # chain completes during the V projection) ----
            pex_all = pex_p.tile([P, NB, NHD, T], BF16, tag="pex")
            for b in range(NB):
                bt0 = b * T
                s_ps = ptile(P, (P, NHD, T), f"s_ps{b % 2}")
                for p_ in range(NHD):
                    nc.tensor.matmul(
                        s_ps[:, p_, :],
                        ktbd[:, p_, b, :],
                        qT[:, p_, bt0:bt0 + T],
                        start=True, stop=True)
                nc.scalar.activation(
                    out=pex_all[:, b], in_=s_ps, func=AF.Exp, scale=EXPSC)
                if b % 2 == 1:
                    # gpsimd: pure-SBUF op, keeps the busy DVE queue clear
                    nc.gpsimd.tensor_tensor(
                        pex_all[:, b - 1:b + 1], pex_all[:, b - 1:b + 1],
                        mask_sb, ALU.mult)
            # ---- P1b: V in natural layout, block-diag via PER-TILE remaps ----
            v_sb = v_p.tile([P, TT, H, Dh], BF16, tag="v_sb")
            v_sb2 = v_sb.rearrange("p a (h two) c -> p a h two c", two=2)
            vbd_v = vbd.rearrange("p a (nb2 two) c -> p a nb2 two c", two=2)
            for it in range(TT):
                psA = ptile(P, (P, 512), "v_psA")
                psB = ptile(P, (P, 256), "v_psB")
                for cc in range(NCC):
                    lhs = xT[:, cc, it * P:(it + 1) * P]
                    nc.tensor.matmul(psA, lhs, wvT[:, cc, 0:512],
                                     start=(cc == 0), stop=(cc == NCC - 1))
                    nc.tensor.matmul(psB, lhs, wvT[:, cc, 512:768],
                                     start=(cc == 0), stop=(cc == NCC - 1))
                nc.scalar.copy(
                    out=v_sb[:, it, 0:8, :],
                    in_=psA.rearrange("p (a b) -> p a b", a=8))
                nc.scalar.copy(
                    out=v_sb[:, it, 8:12, :],
                    in_=psB.rearrange("p (a b) -> p a b", a=4))
                for par in range(2):
                    nc.sync.dma_start(
                        out=vbd_v[0:T, :, it, par, 0:Dh],
                        in_=v_sb2[par * T:(par + 1) * T, it, :, 0, :])
                    nc.sync.dma_start(
                        out=vbd_v[T:P, :, it, par, Dh + 1:2 * Dh + 1],
                        in_=v_sb2[par * T:(par + 1) * T, it, :, 1, :])

            # ---- next chunk X loads (early; cheap SWDGE casting DMAs) ----
            if ci + 1 < n_chunks:
                xbs_next = p0a(ci + 1)

            # ---- P2b: AV + normalization + Y transpose, pipelined per tile
            ytiles = []
            def yt_tr(yb):
                ytA = ptile(P, (P, 4, P), "yt_psA")
                ytB = ptile(P, (P, 2, P), "yt_psB")
                for j in range(6):
                    dst = ytA[:, j, :] if j < 4 else ytB[:, j - 4, :]
                    tr128(dst, yb[:, j * P:(j + 1) * P])
                ytile = yt_p.tile([P, NHD, P], BF16, tag="ytile")
                nc.scalar.copy(out=ytile[:, 0:4, :], in_=ytA)
                nc.scalar.copy(out=ytile[:, 4:6, :], in_=ytB)
                ytiles.append(ytile)
            def oproj_emit(it):
                ytile = ytiles[it]
                oA = ptile(P, (P, 512), "o_psA")
                oB = ptile(P, (P, 256), "o_psB")
                for j in range(NHD):
                    lhs = ytile[:, j, :]
                    nc.tensor.matmul(oA, lhs, wpT[:, j, 0:512],
                                     start=(j == 0), stop=(j == NHD - 1))
                    nc.tensor.matmul(oB, lhs, wpT[:, j, 512:768],
                                     start=(j == 0), stop=(j == NHD - 1))
                osb = ostage.tile([P, C], F32, tag="osb")
                nc.vector.tensor_tensor(osb[:, 0:512], oA, bp_bc[:, 0:512], ALU.add)
                nc.vector.tensor_tensor(osb[:, 512:768], oB, bp_bc[:, 512:768], ALU.add)
                row0 = tok0 + it * P
                nc.sync.dma_start(out=of[row0:row0 + P, :], in_=osb)

            # AV, normalize, Y-transpose and output projection software-
            # pipelined per tile: the long oproj matmul streams keep the PE
            # duty high so the HAM clock-gate never re-throttles mid-chunk
            ybs = []
            for it in range(TT):
                yb = y_p.tile([P, HD], BF16, tag="yb")
                ybs.append(yb)
                y_ps = [ptile(P, (P, 3, 2 * (Dh + 1)), f"y_ps{h2}") for h2 in range(2)]
                for p_ in range(NHD):
                    for half in range(2):      # two batches per 128-token tile
                        b = it * 2 + half
                        prow = half * T
                        nc.tensor.matmul(
                            y_ps[p_ // 3][prow:prow + T, p_ % 3, :],
                            pex_all[:, b, p_, :],
                            vbd[:, p_, b, :],
                            start=True, stop=True)
                for h2 in range(2):
                    y_v = y_ps[h2].rearrange("p a (two c) -> p a two c", c=Dh + 1)
                    rec = small.tile([P, 3, 2, 1], F32, tag="rec", name="rec")
                    nc.vector.reciprocal(out=rec, in_=y_v[:, :, :, Dh:Dh + 1])
                    nc.vector.tensor_tensor(
                        yb[:, h2 * 384:(h2 + 1) * 384]
                            .rearrange("p (a two b) -> p a two b", a=3, two=2),
                        y_v[:, :, :, 0:Dh],
                        rec.to_broadcast([P, 3, 2, Dh]),
                        ALU.mult)
                if it >= 2:
                    yt_tr(ybs[it - 2])
                if it >= 3:
                    oproj_emit(it - 3)
            yt_tr(ybs[TT - 2])
            oproj_emit(TT - 3)
            yt_tr(ybs[TT - 1])
            oproj_emit(TT - 2)
            if ci + 1 < n_chunks:
                xT_next, x8_next = p0b(xbs_next)
            oproj_emit(TT - 1)
            if ci + 1 < n_chunks:
                xT, x8 = xT_next, x8_next

    _split_multi_waits(nc, mybir)
    return nc


def _get_program(B_loc, chunk_tok):
    key = (B_loc, chunk_tok)
    if key not in _cache:
        _cache[key] = build_nc(B_loc, chunk_tok)
    return _cache[key]


def make_const_inputs():
    import ml_dtypes
    ident = np.eye(P, dtype=ml_dtypes.bfloat16)
    # mask[s, t] = 1 if s <= t (causal, scoresT layout)
    m = np.tril(np.ones((T, T), dtype=np.float32)).T
    m2 = np.vstack([m, m])   # replicated for both partition-halves
    mask = np.tile(m2, (1, 2 * NHD)).astype(ml_dtypes.bfloat16)  # [P, 2*NHD*T]
    return ident, mask


def prepare(x, Wq, Wk, Wv, Wp, bp, chunk_tok=1024):
    x = np.ascontiguousarray(x, dtype=np.float32)
    B_loc = B // N_CORES
    ident, mask = make_const_inputs()
    nc = _get_program(B_loc, chunk_tok)
    in_maps = []
    for c in range(N_CORES):
        in_maps.append({
            "x": x[c * B_loc:(c + 1) * B_loc],
            "Wq": np.ascontiguousarray(Wq, dtype=np.float32),
            "Wk": np.ascontiguousarray(Wk, dtype=np.float32),
            "Wv": np.ascontiguousarray(Wv, dtype=np.float32),
            "Wp": np.ascontiguousarray(Wp, dtype=np.float32),
            "bp": np.ascontiguousarray(bp, dtype=np.float32),
            "ident": ident,
            "mask": mask,
        })
    return nc, in_maps


def kernel(x, Wq, Wk, Wv, Wp, bp):
    from concourse import bass_utils

    nc, in_maps = prepare(x, Wq, Wk, Wv, Wp, bp)
    res = bass_utils.run_bass_kernel_spmd(nc, in_maps, list(range(N_CORES)))
    return np.concatenate([res.results[c]["out"] for c in range(N_CORES)], axis=0)



# revision 41
# speedup vs baseline: 1.9547x; 1.0663x over previous
"""Multi-head causal attention (B=1024, T=64, C=768, H=12, D=64) on 8 TRN2
NeuronCores, data-parallel over the batch dimension (128 batches/core).

Dataflow per core (all matmuls bf16, fp32 PSUM accumulate), processed in
1024-token chunks with coarse per-engine phases so the PE streams
back-to-back matmuls (no per-batch PE<->Scalar<->Vector ping-pong) and the
HAM clock-gate stays warm:
  - X tiles are loaded with SWDGE casting-DMAs (f32 DRAM -> bf16 SBUF, one
    chunk ahead) and PE-transposed (regular matmul vs a moving identity)
    into XT [c, tok].
  - P1a: KT then QT [hd, tok] = WT.T @ XT (weights stationary); KT is
    rewritten block-diagonally (2 heads per 128 partitions) into ktbd.
  - P2a immediately after: scoresT[s,t] = ktbd.T @ QT for all 16 batches;
    exp (Scalar, fused 1/8 scale) + causal-mask multiply (Vector) stage
    pex in SBUF while the PE moves on -- the exp/mask chain drains during
    the V projection, so AV never waits.
  - P1b: V in natural [tok, hd] layout, remapped per-tile by DMA into the
    block-diagonal Vaug (extra ones column -> softmax denominator falls out
    of the AV matmul for free).
  - P2b/P3 fused: AV matmuls Y[t, (h,d)|den] = pex.T @ Vaug with the two
    batch halves interleaved (disjoint PE column groups run concurrently);
    normalization (Vector), the Y transpose (PE, lag 2) and the output
    projection out[t, c] = YT.T @ WpT + bp (lag 3) are software-pipelined
    per tile -- the long projection streams keep PE duty high enough that
    the HAM clock-gate never re-throttles mid-chunk.
"""

import numpy as np

P = 128
B, T, C, H, Dh = 1024, 64, 768, 12, 64
HD = H * Dh            # 768
NCC = C // P           # 6 contraction chunks
NHD = HD // P          # 6 hd chunks
N_CORES = 8

_cache = {}


def _patch_tile_drain(tile, mybir):
    """walrus CTRL (Drain) ops in this toolchain accept only 1 sem-wait;
    spread the TileContext exit-drain's waits across preceding SP nops."""
    from concourse.vector_clock import ScopedClock

    if getattr(tile.TileContext, "_drain_patched", False):
        return

    def _drain_and_barrier(self, tick_clock, wait_clock):
        nc = self.nc
        drain_inst = nc.sync.drain()
        wait_clock.add_sem_waits(
            drain_inst.ins, ScopedClock({None: tick_clock.global_clock})
        )
        waits = list(drain_inst.ins.sync_info.on_wait)
        if len(waits) > 1:
            drain_inst.ins.sync_info.on_wait = waits[-1:]
            cur_bb = nc.cur_bb.bb
            idx = cur_bb.instructions.index(drain_inst.ins)
            extra = []
            for w in waits[:-1]:
                nop = mybir.InstNoOp(name=f"I-{nc.next_id()}", ins=[], outs=[])
                nop.engine = drain_inst.ins.engine
                nop.sync_info = mybir.SyncInfo(on_wait=[w], on_update=[])
                nc.register_instruction(nop)
                extra.append(nop)
            cur_bb.instructions[idx:idx] = extra
        nc.all_engine_barrier()
        assert self.sems is not None
        popped = nc._tile_sem_poison_stack.pop()
        assert popped is self._sem_poison
        nc.clear_and_free_semaphores(list(self.sems.allocated().values()))
        nc.all_engine_barrier()

    tile.TileContext._drain_and_barrier = _drain_and_barrier
    tile.TileContext._drain_patched = True


def _install_loud_cc_hook():
    """Surface real exceptions from the neuronx_cc hook (C wrapper eats them)."""
    from concourse import bass2jax as _b2j
    if getattr(_b2j, "_loud_hook_installed", False):
        return
    _orig = _b2j.neuronx_cc_hook
    def _loud(*a, **k):
        try:
            return _orig(*a, **k)
        except BaseException:
            import traceback
            traceback.print_exc()
            raise
    _b2j.neuronx_cc_hook = _loud
    _b2j._loud_hook_installed = True


def _split_multi_waits(nc, mybir, K=1):
    """This walrus build supports only one sem-wait per instruction: move
    excess waits onto same-engine NOPs inserted directly before the owner."""
    def fix_block(bb):
        insts = bb.instructions
        i = 0
        while i < len(insts):
            ins = insts[i]
            si = ins.sync_info
            w = list(si.on_wait) if si is not None and si.on_wait else []
            if len(w) > K:
                carriers = []
                for j in range(0, len(w) - K, K):
                    nop = mybir.InstNoOp(name=f"I-{nc.next_id()}", ins=[], outs=[])
                    nop.engine = ins.engine
                    nop.sync_info = mybir.SyncInfo(on_wait=w[j:j + K], on_update=[])
                    nc.register_instruction(nop)
                    carriers.append(nop)
                si.on_wait = w[len(w) - K:]
                insts[i:i] = carriers
                i += len(carriers)
            i += 1
    for fn in nc.m.functions:
        for bb in fn.blocks:
            fix_block(bb)


def _bp_bcast_ap(bass, bp_d):
    a = bp_d[:]
    return bass.AP(tensor=a.tensor, offset=a.offset, ap=[[0, P]] + list(a.ap))


def build_nc(B_loc=B // N_CORES, chunk_tok=1024, phases=6):
    import concourse.bass as bass
    import concourse.tile as tile
    from concourse import mybir
    from contextlib import ExitStack

    _patch_tile_drain(tile, mybir)
    _install_loud_cc_hook()

    F32 = mybir.dt.float32
    BF16 = mybir.dt.bfloat16
    F8 = mybir.dt.float8e4
    AF = mybir.ActivationFunctionType
    ALU = mybir.AluOpType
    DR = mybir.MatmulPerfMode.DoubleRow

    BT = B_loc * T
    chunk_tok = min(chunk_tok, BT)
    n_chunks = BT // chunk_tok
    assert n_chunks * chunk_tok == BT
    TT = chunk_tok // P     # 128-token tiles per chunk
    NB = chunk_tok // T     # batches per chunk

    # Q/K projections run in fp8(e4m3) DoubleRow perf mode: 2 contraction
    # rows per partition per cycle (halves the QK streaming time). W is
    # pre-scaled by 2^10 into fp8 (keeps 0.02-std weights out of the
    # subnormal range); the 2^-20 descale folds into the exp scale.
    W8SC = 1024.0
    EXPSC = 0.125 * (2.0 ** -20)

    nc = bass.Bass()
    x_d = nc.declare_dram_parameter("x", [B_loc, T, C], F32, isOutput=False)
    wq_d = nc.declare_dram_parameter("Wq", [H, Dh, C], F32, isOutput=False)
    wk_d = nc.declare_dram_parameter("Wk", [H, Dh, C], F32, isOutput=False)
    wv_d = nc.declare_dram_parameter("Wv", [H, Dh, C], F32, isOutput=False)
    wp_d = nc.declare_dram_parameter("Wp", [C, HD], F32, isOutput=False)
    bp_d = nc.declare_dram_parameter("bp", [C], F32, isOutput=False)
    id_d = nc.declare_dram_parameter("ident", [P, P], BF16, isOutput=False)
    mk_d = nc.declare_dram_parameter("mask", [P, 2 * NHD * T], BF16, isOutput=False)
    out_d = nc.declare_dram_parameter("out", [B_loc, T, C], F32, isOutput=True)

    xf = x_d[:].flatten_outer_dims()      # [BT, C]
    of = out_d[:].flatten_outer_dims()    # [BT, C]

    with tile.TileContext(nc) as tc, ExitStack() as ctx:
        sing = ctx.enter_context(tc.tile_pool(name="sing", bufs=1))
        xbf_p = ctx.enter_context(tc.tile_pool(name="xbfp", bufs=8))
        wbf_p = ctx.enter_context(tc.tile_pool(name="wbfp", bufs=3))
        ostage = ctx.enter_context(tc.tile_pool(name="ostage", bufs=2))
        xT_p = ctx.enter_context(tc.tile_pool(name="xTp", bufs=2))
        x8_p = ctx.enter_context(tc.tile_pool(name="x8p", bufs=2))
        qT_p = ctx.enter_context(tc.tile_pool(name="qTp", bufs=1))
        v_p = ctx.enter_context(tc.tile_pool(name="vp", bufs=1))
        pex_p = ctx.enter_context(tc.tile_pool(name="pex", bufs=1))
        y_p = ctx.enter_context(tc.tile_pool(name="y", bufs=4))
        yt_p = ctx.enter_context(tc.tile_pool(name="yt", bufs=6))
        small = ctx.enter_context(tc.tile_pool(name="small", bufs=6))
        pp = ctx.enter_context(tc.tile_pool(name="pp", bufs=8, space="PSUM"))

        def ptile(pdim, shape, name, dt=None):
            # all PSUM tiles share one 1-bank slot class: within a phase this
            # gives maximum in-flight tiles (a per-phase split starves P1a)
            t = pp.tile([P, 512], dt or F32, tag="ps", name=name)
            flat = t[:pdim, : int(np.prod(shape[1:]))]
            return flat.rearrange(
                "p (a b) -> p a b", a=shape[1]
            ) if len(shape) == 3 else flat

        def tr128(dst, src):
            # dst = src.T as a REGULAR matmul (normal LDW + moving identity):
            # cheap on the PE and counts as PE activity for the HAM clock-gate
            nc.tensor.matmul(dst, src, id_sb, start=True, stop=True)

        # ---- constants ----
        id_sb = sing.tile([P, P], BF16)
        nc.sync.dma_start(out=id_sb, in_=id_d[:])
        mask_sb = sing.tile([P, 2, NHD, T], BF16)
        nc.sync.dma_start(out=mask_sb, in_=mk_d[:].rearrange(
            "p (two a b) -> p two a b", two=2, a=NHD))
        bp_bc = sing.tile([P, C], F32)
        nc.gpsimd.dma_start(
            out=bp_bc,
            in_=_bp_bcast_ap(bass, bp_d),
        )

        # ---- persistent block-diagonal attention operands ----
        # (memsets moved off the gpsimd queue -- they were serializing ahead
        # of the weight-DMA dispatches there -- and emitted after the wk/wq
        # loads below so the DMA queue starts immediately)
        NBmax = chunk_tok // T
        ktbd = sing.tile([P, NHD, NBmax, P], BF16, name="ktbd")
        vbd = sing.tile([P, NHD, NBmax, 2 * (Dh + 1)], BF16, name="vbd")

        # ---- weight prep: WT_sb[col_in, col_out, row] = flat[row, col]^T ----
        def prep_wT(wflat, name):
            wT = sing.tile([P, NCC, 768], BF16, name=name)
            evac = nc.scalar.copy if name in ("wvT", "wpT") else nc.vector.tensor_copy
            for r in range(6):
                wbf = wbf_p.tile([P, 768], BF16, tag="wbf")
                # SWDGE casting DMA: f32 DRAM -> bf16 SBUF in one shot
                nc.gpsimd.dma_start(out=wbf, in_=wflat[r * P:(r + 1) * P, :])
                psA = ptile(P, (P, 4, P), f"{name}_psA_{r}")
                psB = ptile(P, (P, 2, P), f"{name}_psB_{r}")
                for j in range(6):
                    dst = psA[:, j, :] if j < 4 else psB[:, j - 4, :]
                    tr128(dst, wbf[:, j * P:(j + 1) * P])
                evac(out=wT[:, 0:4, r * P:(r + 1) * P], in_=psA)
                evac(out=wT[:, 4:6, r * P:(r + 1) * P], in_=psB)
            return wT

        def p0a(ci):
            # casting loads (SWDGE) for chunk ci, emitted one chunk early
            tok0 = ci * chunk_tok
            xbs = []
            for it in range(TT):
                row0 = tok0 + it * P
                xb = xbf_p.tile([P, C], BF16, tag="xbf")
                nc.gpsimd.dma_start(out=xb, in_=xf[row0:row0 + P, :])
                xbs.append(xb)
            return xbs

        def p0b(xbs):
            # PE transposes + vector evac into xT; runs mid-chunk so xT is
            # complete well before the next chunk's projections need it.
            # x8 is the fp8 copy feeding the DoubleRow Q/K projections.
            xT = xT_p.tile([P, NCC, chunk_tok], BF16, tag="xT")
            x8 = x8_p.tile([P, NCC, chunk_tok], F8, tag="x8")
            for it in range(TT):
                psA = ptile(P, (P, 4, P), "x_psA")
                psB = ptile(P, (P, 2, P), "x_psB")
                for j in range(6):
                    dst = psA[:, j, :] if j < 4 else psB[:, j - 4, :]
                    tr128(dst, xbs[it][:, j * P:(j + 1) * P])
                nc.vector.tensor_copy(out=xT[:, 0:4, it * P:(it + 1) * P], in_=psA)
                nc.vector.tensor_copy(out=xT[:, 4:6, it * P:(it + 1) * P], in_=psB)
                nc.vector.tensor_copy(
                    out=x8[:, :, it * P:(it + 1) * P],
                    in_=xT[:, :, it * P:(it + 1) * P])
            return xT, x8

        def cast_w8(wT, name):
            w8 = sing.tile([P, NCC, 768], F8, name=name)
            for cc in range(NCC):
                nc.vector.tensor_scalar_mul(w8[:, cc], wT[:, cc], W8SC)
            return w8

        # prologue order follows the first chunk's dependency chain: the
        # K projection needs wkT + xT(0) first, Q next; wv/wp can trail.
        # ktbd zeros ride DVE while the first weight DMA is in flight; vbd
        # zeros ride gpsimd after all prologue DMA dispatches (first vbd
        # write is the chunk-0 V remap, ~35us in).
        nc.vector.memset(ktbd, 0.0)
        wkT = prep_wT(wk_d[:].flatten_outer_dims(), "wkT")
        wk8 = cast_w8(wkT, "wk8")
        wqT = prep_wT(wq_d[:].flatten_outer_dims(), "wqT")
        wq8 = cast_w8(wqT, "wq8")
        xbs0 = p0a(0)
        xT, x8 = p0b(xbs0)
        # wv/wp prep trails chunk 0's X pipeline (their evacs ride Scalar)
        wvT = prep_wT(wv_d[:].flatten_outer_dims(), "wvT")
        wpT = prep_wT(wp_d[:], "wpT")
        nc.gpsimd.memset(vbd, 0.0)
        nc.gpsimd.memset(vbd[0:T, :, :, Dh:Dh + 1], 1.0)
        nc.gpsimd.memset(vbd[T:P, :, :, 2 * Dh + 1:2 * Dh + 2], 1.0)
        for ci in range(n_chunks):
            tok0 = ci * chunk_tok

            # ---- P1a: QT / KT projections (weights stationary) ----
            SUBW = min(512, chunk_tok)
            n_sub = chunk_tok // SUBW
            qT = qT_p.tile([P, NHD, chunk_tok], BF16, tag="qT")
            nbsub = SUBW // T          # batches per SUBW token block
            for w8t, dst in ((wk8, "k"), (wq8, "q")):
                for m in range(NHD):
                    pss = [ptile(P, (P, SUBW), f"proj_{m}_{s}") for s in range(n_sub)]
                    for cc in range(NCC // 2):
                        lhs = w8t[:, 2 * cc:2 * cc + 2, m * P:(m + 1) * P]
                        for s in range(n_sub):
                            nc.tensor.matmul(
                                pss[s], lhs,
                                x8[:, 2 * cc:2 * cc + 2, s * SUBW:(s + 1) * SUBW],
                                start=(cc == 0), stop=(cc == NCC // 2 - 1),
                                perf_mode=DR)
                    for s in range(n_sub):
                        if dst == "q":
                            nc.vector.tensor_copy(
                                out=qT[:, m, s * SUBW:(s + 1) * SUBW], in_=pss[s])
                        else:
                            b0 = s * nbsub
                            nc.scalar.copy(
                                out=ktbd[0:T, m, b0:b0 + nbsub, 0:T],
                                in_=pss[s][0:T].rearrange(
                                    "p (nb t) -> p nb t", nb=nbsub))
                            nc.scalar.copy(
                                out=ktbd[T:P, m, b0:b0 + nbsub, T:P],
                                in_=pss[s][T:P].rearrange(
                                    "p (nb t) -> p nb t", nb=nbsub))
            # ---- P2a+P1b interleaved per tile: scores+exp+mask | V proj.
            # The Scalar exp for a tile's two batches drains while the PE
            # streams that tile's V projection, so exp never gates the PSUM
            # recycle and AV never waits.
            pex_all = pex_p.tile([P, NB, NHD, T], BF16, tag="pex")
            v_sb = v_p.tile([P, TT, H, Dh], BF16, tag="v_sb")
            v_sb2 = v_sb.rearrange("p a (h two) c -> p a h two c", two=2)
            vbd_v = vbd.rearrange("p a (nb2 two) c -> p a nb2 two c", two=2)
            for it in range(TT):
                for half in range(2):
                    b = 2 * it + half
                    s_ps = ptile(P, (P, NHD, T), f"s_ps{b % 2}")
                    for p_ in range(NHD):
                        nc.tensor.matmul(
                            s_ps[:, p_, :],
                            ktbd[:, p_, b, :],
                            qT[:, p_, b * T:(b + 1) * T],
                            start=True, stop=True)
                    nc.scalar.activation(
                        out=pex_all[:, b], in_=s_ps, func=AF.Exp, scale=EXPSC)
                # gpsimd: pure-SBUF op, keeps the busy DVE queue clear
                nc.gpsimd.tensor_tensor(
                    pex_all[:, 2 * it:2 * it + 2], pex_all[:, 2 * it:2 * it + 2],
                    mask_sb, ALU.mult)
                psA = ptile(P, (P, 512), "v_psA")
                psB = ptile(P, (P, 256), "v_psB")
                for cc in range(NCC):
                    lhs = xT[:, cc, it * P:(it + 1) * P]
                    nc.tensor.matmul(psA, lhs, wvT[:, cc, 0:512],
                                     start=(cc == 0), stop=(cc == NCC - 1))
                    nc.tensor.matmul(psB, lhs, wvT[:, cc, 512:768],
                                     start=(cc == 0), stop=(cc == NCC - 1))
                nc.scalar.copy(
                    out=v_sb[:, it, 0:8, :],
                    in_=psA.rearrange("p (a b) -> p a b", a=8))
                nc.scalar.copy(
                    out=v_sb[:, it, 8:12, :],
                    in_=psB.rearrange("p (a b) -> p a b", a=4))
                for par in range(2):
                    nc.sync.dma_start(
                        out=vbd_v[0:T, :, it, par, 0:Dh],
                        in_=v_sb2[par * T:(par + 1) * T, it, :, 0, :])
                    nc.sync.dma_start(
                        out=vbd_v[T:P, :, it, par, Dh + 1:2 * Dh + 1],
                        in_=v_sb2[par * T:(par + 1) * T, it, :, 1, :])

            # ---- next chunk X loads (early; cheap SWDGE casting DMAs) ----
            if ci + 1 < n_chunks:
                xbs_next = p0a(ci + 1)

            # ---- P2b: AV + normalization + Y transpose, pipelined per tile
            ytiles = []
            def yt_tr(yb):
                ytA = ptile(P, (P, 4, P), "yt_psA")
                ytB = ptile(P, (P, 2, P), "yt_psB")
                for j in range(6):
                    dst = ytA[:, j, :] if j < 4 else ytB[:, j - 4, :]
                    tr128(dst, yb[:, j * P:(j + 1) * P])
                ytile = yt_p.tile([P, NHD, P], BF16, tag="ytile")
                nc.scalar.copy(out=ytile[:, 0:4, :], in_=ytA)
                nc.scalar.copy(out=ytile[:, 4:6, :], in_=ytB)
                ytiles.append(ytile)
            def oproj_emit(it):
                ytile = ytiles[it]
                oA = ptile(P, (P, 512), "o_psA")
                oB = ptile(P, (P, 256), "o_psB")
                for j in range(NHD):
                    lhs = ytile[:, j, :]
                    nc.tensor.matmul(oA, lhs, wpT[:, j, 0:512],
                                     start=(j == 0), stop=(j == NHD - 1))
                    nc.tensor.matmul(oB, lhs, wpT[:, j, 512:768],
                                     start=(j == 0), stop=(j == NHD - 1))
                osb = ostage.tile([P, C], F32, tag="osb")
                nc.vector.tensor_tensor(osb[:, 0:512], oA, bp_bc[:, 0:512], ALU.add)
                nc.vector.tensor_tensor(osb[:, 512:768], oB, bp_bc[:, 512:768], ALU.add)
                row0 = tok0 + it * P
                nc.sync.dma_start(out=of[row0:row0 + P, :], in_=osb)

            # AV, normalize, Y-transpose and output projection software-
            # pipelined per tile: the long oproj matmul streams keep the PE
            # duty high so the HAM clock-gate never re-throttles mid-chunk
            ybs = []
            for it in range(TT):
                yb = y_p.tile([P, HD], BF16, tag="yb")
                ybs.append(yb)
                y_ps = [ptile(P, (P, 3, 2 * (Dh + 1)), f"y_ps{h2}") for h2 in range(2)]
                for p_ in range(NHD):
                    for half in range(2):      # two batches per 128-token tile
                        b = it * 2 + half
                        prow = half * T
                        nc.tensor.matmul(
                            y_ps[p_ // 3][prow:prow + T, p_ % 3, :],
                            pex_all[:, b, p_, :],
                            vbd[:, p_, b, :],
                            start=True, stop=True)
                for h2 in range(2):
                    y_v = y_ps[h2].rearrange("p a (two c) -> p a two c", c=Dh + 1)
                    rec = small.tile([P, 3, 2, 1], F32, tag="rec", name="rec")
                    nc.vector.reciprocal(out=rec, in_=y_v[:, :, :, Dh:Dh + 1])
                    nc.vector.tensor_tensor(
                        yb[:, h2 * 384:(h2 + 1) * 384]
                            .rearrange("p (a two b) -> p a two b", a=3, two=2),
                        y_v[:, :, :, 0:Dh],
                        rec.to_broadcast([P, 3, 2, Dh]),
                        ALU.mult)
                if it >= 2:
                    yt_tr(ybs[it - 2])
                if it >= 3:
                    oproj_emit(it - 3)
            yt_tr(ybs[TT - 2])
            oproj_emit(TT - 3)
            yt_tr(ybs[TT - 1])
            oproj_emit(TT - 2)
            if ci + 1 < n_chunks:
                xT_next, x8_next = p0b(xbs_next)
            oproj_emit(TT - 1)
            if ci + 1 < n_chunks:
                xT, x8 = xT_next, x8_next

    _split_multi_waits(nc, mybir)
    return nc


def _get_program(B_loc, chunk_tok):
    key = (B_loc, chunk_tok)
    if key not in _cache:
        _cache[key] = build_nc(B_loc, chunk_tok)
    return _cache[key]


def make_const_inputs():
    import ml_dtypes
    ident = np.eye(P, dtype=ml_dtypes.bfloat16)
    # mask[s, t] = 1 if s <= t (causal, scoresT layout)
    m = np.tril(np.ones((T, T), dtype=np.float32)).T
    m2 = np.vstack([m, m])   # replicated for both partition-halves
    mask = np.tile(m2, (1, 2 * NHD)).astype(ml_dtypes.bfloat16)  # [P, 2*NHD*T]
    return ident, mask


def prepare(x, Wq, Wk, Wv, Wp, bp, chunk_tok=1024):
    x = np.ascontiguousarray(x, dtype=np.float32)
    B_loc = B // N_CORES
    ident, mask = make_const_inputs()
    nc = _get_program(B_loc, chunk_tok)
    in_maps = []
    for c in range(N_CORES):
        in_maps.append({
            "x": x[c * B_loc:(c + 1) * B_loc],
            "Wq": np.ascontiguousarray(Wq, dtype=np.float32),
            "Wk": np.ascontiguousarray(Wk, dtype=np.float32),
            "Wv": np.ascontiguousarray(Wv, dtype=np.float32),
            "Wp": np.ascontiguousarray(Wp, dtype=np.float32),
            "bp": np.ascontiguousarray(bp, dtype=np.float32),
            "ident": ident,
            "mask": mask,
        })
    return nc, in_maps


def kernel(x, Wq, Wk, Wv, Wp, bp):
    from concourse import bass_utils

    nc, in_maps = prepare(x, Wq, Wk, Wv, Wp, bp)
    res = bass_utils.run_bass_kernel_spmd(nc, in_maps, list(range(N_CORES)))
    return np.concatenate([res.results[c]["out"] for c in range(N_CORES)], axis=0)

